# revision 34
# baseline (speedup 1.0000x reference)
"""DRASI encoder (MLP -> GraphConv x2 -> mu/logvar heads) on 8 Trainium2 cores.

Sharding: nodes are split into 8 contiguous shards of 6250. Each core runs the
node-local MLP on its shard (transposed layout, weights as matmul lhsT), the
shards are AllGathered into a full [50000, 128] bf16 feature table in DRAM, and
each core processes the edges whose destination lies in its shard:

  - edges are sorted by dst and bucketed into 96-node "groups"; each
    (group, src-half) bucket is padded to whole 128-edge blocks, with the
    block count unified across cores (max) so all 8 cores share one program;
  - groups are packed into large chunks (<= 96 blocks); one dma_gather per
    (chunk, src-half) fetches source rows from the table (int16 indices, so
    the table is addressed as two 25000-row halves);
  - the selection matrix S_w[e, s] = w_e * (seg_e == s) is precomputed on the
    host in bf16 and streamed in via plain DMA (which does not contend with
    the gather path), so no on-device one-hot build is needed;
  - per-group PE matmuls accumulate aggT = msg.T @ S_w in PSUM and evict to
    an SBUF table in bf16;
  - the GraphConv root-term matmuls run during the AllGather windows into a
    bf16 buffer and are re-injected into the rel-term PSUM accumulation with
    an identity matmul, so the linear phase is one PSUM group + one
    bias+relu activation per 512-column tile, interleaved with aggregation;
  - linear outputs are PE-transposed back to natural layout and published
    with two strided DMAs per AllGather table.

Outputs (mu, logvar) are computed per shard and concatenated on the host.
"""
import sys
sys.path.insert(0, '/opt/trn_rl_repo')

import numpy as np
import concourse.bass as bass
import concourse.bacc as bacc
import concourse.mybir as mybir
from concourse.tile import TileContext
from concourse.masks import make_identity
from concourse import bass_utils

P = 128
N_CORES = 8
N_NODES = 50000
IN_DIM = 512
HID = 128
LAT = 32
SHARD = N_NODES // N_CORES          # 6250
HALF = N_NODES // 2                 # 25000
W = 96                              # nodes per segment group (PSUM tile width)
MAXBLK = 72                         # max 128-edge blocks per chunk
N_GROUPS = (SHARD + W - 1) // W     # 66
N_TILES = [512] * (SHARD // 512) + ([SHARD % 512] if SHARD % 512 else [])
F32 = mybir.dt.float32
BF16 = mybir.dt.bfloat16
I16 = mybir.dt.int16
import ml_dtypes
NP_BF16 = ml_dtypes.bfloat16


# ---------------------------------------------------------------- host prep --

def _unified_structure(per_core_edges):
    """per_core_edges: list of (src, dst_local, w) sorted by dst_local.
    Returns (chunk_meta, per-core idx arrays, per-core S_w arrays)."""
    buckets = [[[None, None] for _ in range(N_GROUPS)] for _ in range(N_CORES)]
    for c, (src, dstl, wgt) in enumerate(per_core_edges):
        grp = dstl // W
        for g in range(N_GROUPS):
            sel = grp == g
            gs, gd, gw = src[sel], dstl[sel], wgt[sel]
            hi = gs >= HALF
            for h in (0, 1):
                m = hi == bool(h)
                buckets[c][g][h] = (gs[m] - h * HALF, gd[m] - g * W, gw[m])

    # unified block count per (group, half): max over cores, >= 1 block per
    # group total so every group gets an eviction
    B = np.zeros((N_GROUPS, 2), np.int64)
    for g in range(N_GROUPS):
        for h in (0, 1):
            B[g, h] = max((buckets[c][g][h][0].shape[0] + P - 1) // P
                          for c in range(N_CORES))
        if B[g, 0] == 0 and B[g, 1] == 0:
            B[g, 0] = 1

    # pack consecutive groups into chunks of <= MAXBLK blocks (lo and hi
    # share one msg tile: lo runs occupy blocks [0, nblk_lo), hi the rest).
    # The first chunks are small so the first segment matmuls start early,
    # and the final chunk is small to shorten the conv tail.
    caps = [24, 48]
    chunks = []
    cur, cur_n = [], 0
    for g in range(N_GROUPS):
        nb = int(B[g, 0] + B[g, 1])
        cap = caps[len(chunks)] if len(chunks) < len(caps) else MAXBLK
        if cur and cur_n + nb > cap:
            chunks.append(cur)
            cur, cur_n = [], 0
        cur.append(g)
        cur_n += nb
    if cur:
        chunks.append(cur)
    # rebalance the tail: if the last chunk is large, split it
    if len(chunks) >= 2:
        last = chunks[-1]
        nlast = sum(int(B[g, 0] + B[g, 1]) for g in last)
        if nlast > 32 and len(last) >= 2:
            csum = 0
            for i in range(len(last) - 1, -1, -1):
                csum += int(B[last[i], 0] + B[last[i], 1])
                if csum >= 16:
                    break
            if i > 0:
                chunks[-1] = last[:i]
                chunks.append(last[i:])

    chunk_meta = []
    core_idx = [[] for _ in range(N_CORES)]
    core_sw = [[] for _ in range(N_CORES)]
    for groups in chunks:
        nblk_lo = int(sum(B[g, 0] for g in groups))
        nblk_hi = int(sum(B[g, 1] for g in groups))
        nblk = nblk_lo + nblk_hi
        runs = []
        b = 0
        for h in (0, 1):
            for g in groups:
                nb = int(B[g, h])
                if nb:
                    runs.append((g, h, b, b + nb))
                    b += nb
        chunk_meta.append(dict(nblk=nblk, nblk_lo=nblk_lo, runs=runs,
                               groups=list(groups)))

        for c in range(N_CORES):
            idx_flat = np.zeros(nblk * P, np.int16)
            seg_flat = np.full(nblk * P, -1.0, np.float32)
            w_flat = np.zeros(nblk * P, np.float32)
            for (g, h, b0, b1_) in runs:
                ids, segs, ws = buckets[c][g][h]
                n = ids.shape[0]
                o = b0 * P
                idx_flat[o:o + n] = ids.astype(np.int16)
                seg_flat[o:o + n] = segs.astype(np.float32)
                w_flat[o:o + n] = ws
            idx_t = np.tile(idx_flat.reshape(nblk * 8, 16).T, (8, 1))
            core_idx[c].append(idx_t)
            # S_w[e, s, b] = w * (seg == s); padded slots have seg=-1 -> zero
            seg_b = seg_flat.reshape(nblk, P)          # [b, e]
            w_b = w_flat.reshape(nblk, P)
            sw = (seg_b[:, :, None] == np.arange(W)[None, None, :])
            sw = sw * w_b[:, :, None]                  # [b, e, s]
            # device tile layout: [e(part), b, s]
            sw_t = np.ascontiguousarray(
                sw.transpose(1, 0, 2).astype(NP_BF16)) # [e, b, s]
            core_sw[c].append(sw_t.reshape(P, W * nblk))

    eidx = [np.ascontiguousarray(np.concatenate(core_idx[c], axis=1))
            for c in range(N_CORES)]
    esw = [np.ascontiguousarray(np.concatenate(core_sw[c], axis=1))
           for c in range(N_CORES)]
    return chunk_meta, eidx, esw


# ------------------------------------------------------------- device build --

def _build(metas, idx_cols, sw_cols):
    nc = bacc.Bacc(None, target_bir_lowering=False, num_devices=N_CORES,
                   num_swdge_queues=2)

    xT = nc.dram_tensor("xT", [IN_DIM, SHARD], BF16, kind="ExternalInput")
    w1T = nc.dram_tensor("w1T", [IN_DIM, HID], BF16, kind="ExternalInput")
    b1 = nc.dram_tensor("b1", [HID, 1], F32, kind="ExternalInput")
    w2T = nc.dram_tensor("w2T", [HID, HID], BF16, kind="ExternalInput")
    b2 = nc.dram_tensor("b2", [HID, 1], F32, kind="ExternalInput")
    conv_wT = nc.dram_tensor("conv_wT", [2, 2, HID, HID], BF16, kind="ExternalInput")
    conv_b = nc.dram_tensor("conv_b", [2, HID, 1], F32, kind="ExternalInput")
    headWT = nc.dram_tensor("headWT", [HID, 2 * LAT], BF16, kind="ExternalInput")
    head_b = nc.dram_tensor("head_b", [2 * LAT, 1], F32, kind="ExternalInput")
    eidx = nc.dram_tensor("eidx", [P, idx_cols], I16, kind="ExternalInput")
    esw = nc.dram_tensor("esw", [P, sw_cols], BF16, kind="ExternalInput")
    muv_out = nc.dram_tensor("muvT", [2 * LAT, SHARD], F32, kind="ExternalOutput")

    ag_in = [nc.dram_tensor(f"ag_in{i}", [SHARD, HID], BF16) for i in range(2)]
    tables = [nc.dram_tensor(f"h_full{i}", [N_NODES, HID], BF16,
                             addr_space="Shared") for i in range(2)]

    NT_FULL = SHARD // P            # 48 full 128-row publish tiles
    TAIL = SHARD - NT_FULL * P      # 106

    with TileContext(nc) as tc:
        with (
            tc.tile_pool(name="const", bufs=1) as cp,
            tc.tile_pool(name="big", bufs=1) as bigp,
            tc.tile_pool(name="work", bufs=3) as wp,
            tc.tile_pool(name="msgp", bufs=2) as msgp,
            tc.tile_pool(name="ps_lin", bufs=3, space="PSUM") as ps_lin,
            tc.tile_pool(name="ps_tr", bufs=2, space="PSUM") as ps_tr,
            tc.tile_pool(name="ps_agg", bufs=3, space="PSUM") as ps_agg,
        ):
            # ---- constants: keep SP free for x loads; weights go on Act
            w1t_sb = [cp.tile([P, HID], BF16, tag=f"w1_{k}", name=f"w1t_{k}") for k in range(4)]
            for k in range(4):
                nc.scalar.dma_start(out=w1t_sb[k][:], in_=w1T[k * P:(k + 1) * P, :])
            b1_sb = cp.tile([P, 1], F32, tag="b1")
            nc.scalar.dma_start(out=b1_sb[:], in_=b1[:, :])
            w2t_sb = cp.tile([P, HID], BF16, tag="w2")
            nc.scalar.dma_start(out=w2t_sb[:], in_=w2T[:, :])
            b2_sb = cp.tile([P, 1], F32, tag="b2")
            nc.scalar.dma_start(out=b2_sb[:], in_=b2[:, :])
            cw_sb = [[cp.tile([P, HID], BF16, tag=f"cw{l}{m}", name=f"cw_{l}_{m}") for m in range(2)]
                     for l in range(2)]
            for l in range(2):
                for m in range(2):
                    nc.scalar.dma_start(out=cw_sb[l][m][:], in_=conv_wT[l, m, :, :])
            cb_sb = [cp.tile([P, 1], F32, tag=f"cb{l}", name=f"cb_{l}") for l in range(2)]
            for l in range(2):
                nc.scalar.dma_start(out=cb_sb[l][:], in_=conv_b[l, :, :])
            hw_sb = cp.tile([P, 2 * LAT], BF16, tag="hw")
            nc.scalar.dma_start(out=hw_sb[:], in_=headWT[:, :])
            hb_sb = cp.tile([2 * LAT, 1], F32, tag="hb")
            nc.scalar.dma_start(out=hb_sb[:], in_=head_b[:, :])
            ident = cp.tile([P, P], BF16, tag="ident")
            make_identity(nc, ident[:])

            hA = bigp.tile([P, SHARD], BF16, tag="hA")   # h2T, then h4T
            hB = bigp.tile([P, SHARD], BF16, tag="hB")   # h3T
            aggT = bigp.tile([P, SHARD], BF16, tag="aggT")
            rootT_sb = bigp.tile([P, SHARD], BF16, tag="rootT")
            rootT = [rootT_sb, rootT_sb]
            natf = [bigp.tile([P, NT_FULL, HID], BF16, tag=f"natf{i}",
                              name=f"natf_{i}") for i in range(2)]
            natt = [bigp.tile([P, HID], BF16, tag=f"natt{i}",
                              name=f"natt_{i}") for i in range(2)]
            muvT = bigp.tile([2 * LAT, SHARD], F32, tag="muvT")

            def emit_transpose_tiles(hT_tile, t_idx, n0, n1, eng_flip=None):
                # transpose hT[:, n0:n1] into natural-layout staging tiles;
                # PSUM->SBUF eviction on DVE (Act is busy with relus/evicts)
                t = n0 // P
                while n0 < n1:
                    w_ = min(P, n1 - n0)
                    tr_ps = ps_tr.tile([P, P], BF16, space="PSUM", tag="tr",
                                       name="trp")
                    nc.tensor.transpose(out=tr_ps[:w_, :],
                                        in_=hT_tile[:, n0:n0 + w_],
                                        identity=ident[:])
                    dst = natf[t_idx][:, t, :] if w_ == P else natt[t_idx][:TAIL, :]
                    nc.vector.tensor_copy(
                        out=dst[:w_, :] if w_ == P else dst,
                        in_=tr_ps[:w_, :])
                    n0 += w_
                    t += 1

            def emit_publish(t_idx):
                nc.sync.dma_start(
                    out=ag_in[t_idx][:NT_FULL * P, :].rearrange(
                        "(t r) h -> r t h", r=P),
                    in_=natf[t_idx][:, :, :])
                nc.sync.dma_start(out=ag_in[t_idx][NT_FULL * P:, :],
                                  in_=natt[t_idx][:TAIL, :])

            def emit_allgather(t_idx):
                nc.gpsimd.collective_compute(
                    "AllGather", mybir.AluOpType.bypass,
                    replica_groups=[list(range(N_CORES))],
                    ins=[ag_in[t_idx][:, :]],
                    outs=[tables[t_idx][:, :]],
                )

            # ---- MLP (bf16 matmuls, f32 psum) ----
            # software-pipelined so the PE stream has no dependency gaps:
            # stage t emits layer-1 matmuls for tile t, layer-2 for t-1, and
            # publish transposes for t-2
            cols = [sum(N_TILES[:i]) for i in range(len(N_TILES))]
            h1_sbs = {}

            def mlp_l1(t):
                nt, col = N_TILES[t], cols[t]
                # spread the 4 partition chunks over SP(2), Act(1), Pool(1)
                xt2 = wp.tile([P, 2, 512], BF16, tag="xt2")
                nc.sync.dma_start(
                    out=xt2[:, :, :nt],
                    in_=xT[0:2 * P, col:col + nt].rearrange(
                        "(k p) n -> p k n", p=P))
                xta = wp.tile([P, 512], BF16, tag="xta")
                nc.scalar.dma_start(out=xta[:, :nt],
                                    in_=xT[2 * P:3 * P, col:col + nt])
                xt1 = wp.tile([P, 512], BF16, tag="xt1")
                nc.gpsimd.dma_start(out=xt1[:, :nt],
                                    in_=xT[3 * P:4 * P, col:col + nt])
                h1_ps = ps_lin.tile([P, 512], F32, space="PSUM", tag="lin")
                for k in range(4):
                    rhs = (xt2[:, 0, :nt], xt2[:, 1, :nt],
                           xta[:, :nt], xt1[:, :nt])[k]
                    nc.tensor.matmul(out=h1_ps[:, :nt], lhsT=w1t_sb[k][:],
                                     rhs=rhs,
                                     start=(k == 0), stop=(k == 3))
                h1_sb = wp.tile([P, 512], BF16, tag="h1")
                nc.scalar.activation(out=h1_sb[:, :nt], in_=h1_ps[:, :nt],
                                     func=mybir.ActivationFunctionType.Relu,
                                     bias=b1_sb[:])
                h1_sbs[t] = h1_sb

            def mlp_l2(t):
                nt, col = N_TILES[t], cols[t]
                h2_ps = ps_lin.tile([P, 512], F32, space="PSUM", tag="lin")
                nc.tensor.matmul(out=h2_ps[:, :nt], lhsT=w2t_sb[:],
                                 rhs=h1_sbs.pop(t)[:, :nt],
                                 start=True, stop=True)
                nc.scalar.activation(out=hA[:, col:col + nt], in_=h2_ps[:, :nt],
                                     func=mybir.ActivationFunctionType.Relu,
                                     bias=b2_sb[:])

            NTI = len(N_TILES)
            for t in range(NTI + 2):
                if t < NTI:
                    mlp_l1(t)
                if 1 <= t <= NTI:
                    mlp_l2(t - 1)
                if t >= 2:
                    emit_transpose_tiles(hA, 0, cols[t - 2],
                                         cols[t - 2] + N_TILES[t - 2])
            emit_publish(0)
            emit_allgather(0)

            def emit_root(layer, hT_in):
                col = 0
                for nt in N_TILES:
                    ps = ps_lin.tile([P, 512], F32, space="PSUM", tag="lin")
                    nc.tensor.matmul(out=ps[:, :nt], lhsT=cw_sb[layer][1][:],
                                     rhs=hT_in[:, col:col + nt],
                                     start=True, stop=True)
                    nc.scalar.activation(out=rootT[layer][:, col:col + nt],
                                         in_=ps[:, :nt],
                                         func=mybir.ActivationFunctionType.Copy)
                    col += nt

            def conv_layer(layer, hT_in, hT_out, table, pub_idx=None,
                           tile_tail=None):
                # aggregation: chunked gathers + per-group PE segment sums
                icol = 0
                scol = 0
                done_g = 0          # groups fully evicted so far
                done_tiles = 0      # linear tiles emitted so far

                def emit_ready_linear(done_g, done_tiles, force=False):
                    # emit linear tiles whose agg columns are complete
                    avail = min(done_g * W, SHARD)
                    col = done_tiles * 512
                    while done_tiles < len(N_TILES):
                        nt = N_TILES[done_tiles]
                        if col + nt > avail and not force:
                            break
                        ps = ps_lin.tile([P, 512], F32, space="PSUM", tag="lin")
                        nc.tensor.matmul(out=ps[:, :nt], lhsT=cw_sb[layer][0][:],
                                         rhs=aggT[:, col:col + nt],
                                         start=True, stop=False)
                        nc.tensor.matmul(out=ps[:, :nt], lhsT=ident[:],
                                         rhs=rootT[layer][:, col:col + nt],
                                         start=False, stop=True)
                        nc.scalar.activation(
                            out=hT_out[:, col:col + nt], in_=ps[:, :nt],
                            func=mybir.ActivationFunctionType.Relu,
                            bias=cb_sb[layer][:])
                        if pub_idx is not None:
                            emit_transpose_tiles(hT_out, pub_idx, col, col + nt)
                        if tile_tail is not None:
                            tile_tail(col, nt)
                        col += nt
                        done_tiles += 1
                    return done_tiles

                for ci, meta in enumerate(metas):
                    nblk, nblk_lo = meta["nblk"], meta["nblk_lo"]
                    qn = 0
                    idx_t = wp.tile([P, MAXBLK * 8], I16, tag="eidx")
                    nc.scalar.dma_start(out=idx_t[:, :nblk * 8],
                                        in_=eidx[:, icol:icol + nblk * 8])
                    # S_w is a large transfer; DMAs hold the issuing engine,
                    # so it lives on SP (idle during the conv phase)
                    s_w = msgp.tile([P, MAXBLK, W], BF16, tag="sw")
                    nc.sync.dma_start(
                        out=s_w[:, :nblk, :],
                        in_=esw[:, scol:scol + W * nblk].rearrange(
                            "p (b s) -> p b s", s=W))

                    msg = msgp.tile([P, MAXBLK, HID], BF16, tag="msg")
                    if nblk_lo:
                        nc.gpsimd.dma_gather(
                            out_ap=msg[:, :nblk_lo, :], in_ap=table[:HALF, :],
                            idxs_ap=idx_t[:, :nblk_lo * 8],
                            num_idxs=nblk_lo * P, num_idxs_reg=nblk_lo * P,
                            elem_size=HID, single_packet=False,
                            queue_num=qn)
                    if nblk - nblk_lo:
                        nh = nblk - nblk_lo
                        nc.gpsimd.dma_gather(
                            out_ap=msg[:, nblk_lo:nblk, :], in_ap=table[HALF:, :],
                            idxs_ap=idx_t[:, nblk_lo * 8:nblk * 8],
                            num_idxs=nh * P, num_idxs_reg=nh * P,
                            elem_size=HID, single_packet=False,
                            queue_num=qn)

                    # one psum + one eviction per group: a group's lo and hi
                    # runs accumulate into the same tile
                    by_group = {}
                    for (g, h, b0, b1_) in meta["runs"]:
                        by_group.setdefault(g, []).append((h, b0, b1_))
                    for g in meta["groups"]:
                        ps = ps_agg.tile([P, W], F32, space="PSUM", tag="agg")
                        blocks = [(h, b) for (h, b0, b1_) in by_group[g]
                                  for b in range(b0, b1_)]
                        for i, (h, b) in enumerate(blocks):
                            nc.tensor.matmul(out=ps[:], lhsT=msg[:, b, :],
                                             rhs=s_w[:, b, :],
                                             start=(i == 0),
                                             stop=(i == len(blocks) - 1))
                        gw = min(W, SHARD - g * W)
                        nc.scalar.activation(
                            out=aggT[:, g * W:g * W + gw], in_=ps[:, :gw],
                            func=mybir.ActivationFunctionType.Copy)
                    icol += nblk * 8
                    scol += W * nblk
                    done_g += len(meta["groups"])
                    done_tiles = emit_ready_linear(done_g, done_tiles)
                done_tiles = emit_ready_linear(done_g, done_tiles, force=True)
                if pub_idx is not None:
                    emit_publish(pub_idx)

            emit_root(0, hA)
            conv_layer(0, hA, hB, tables[0], pub_idx=1)
            emit_allgather(1)
            emit_root(1, hB)

            # ---- heads fused into conv2's linear phase ----
            def head_tail(col, nt):
                ps = ps_lin.tile([2 * LAT, 512], F32, space="PSUM", tag="lin",
                                 name="headps")
                nc.tensor.matmul(out=ps[:, :nt], lhsT=hw_sb[:],
                                 rhs=hA[:, col:col + nt], start=True, stop=True)
                nc.vector.tensor_tensor(
                    out=muvT[:, col:col + nt], in0=ps[:, :nt],
                    in1=hb_sb[:].to_broadcast([2 * LAT, nt]),
                    op=mybir.AluOpType.add)

            conv_layer(1, hB, hA, tables[1], tile_tail=head_tail)
            HS = SHARD // 2
            nc.sync.dma_start(out=muv_out[:, :HS], in_=muvT[:, :HS])
            nc.scalar.dma_start(out=muv_out[:, HS:], in_=muvT[:, HS:])

    nc.finalize()
    return nc


# -------------------------------------------------------------------- driver --

def _get_compiled(x, edge_index, edge_attr, weights):
    src = np.asarray(edge_index[0]).astype(np.int64)
    dst = np.asarray(edge_index[1]).astype(np.int64)
    wgt = np.asarray(edge_attr, dtype=np.float32)
    x = np.asarray(x, dtype=np.float32)

    per_core_edges = []
    for c in range(N_CORES):
        sel = (dst >= c * SHARD) & (dst < (c + 1) * SHARD)
        s, d, wv = src[sel], dst[sel] - c * SHARD, wgt[sel]
        order = np.argsort(d, kind="stable")
        per_core_edges.append((s[order], d[order], wv[order]))

    metas, eidx, esw = _unified_structure(per_core_edges)
    idx_cols = sum(m["nblk"] * 8 for m in metas)
    sw_cols = sum(m["nblk"] * W for m in metas)

    nc = _build(metas, idx_cols, sw_cols)

    (W1, b1, W2, b2, g1_rel_W, g1_rel_b, g1_root_W,
     g2_rel_W, g2_rel_b, g2_root_W, mu_W, mu_b, lv_W, lv_b) = [
        np.asarray(w, dtype=np.float32) for w in weights]

    conv_wT = np.stack([
        np.stack([g1_rel_W.T, g1_root_W.T]),
        np.stack([g2_rel_W.T, g2_root_W.T]),
    ]).astype(NP_BF16).copy()
    conv_b = np.stack([g1_rel_b[:, None], g2_rel_b[:, None]]).copy()
    headWT = np.ascontiguousarray(
        np.concatenate([mu_W, lv_W], axis=0).T.astype(NP_BF16))
    head_b = np.concatenate([mu_b, lv_b])[:, None].copy()

    common = dict(
        w1T=np.ascontiguousarray(W1.T.astype(NP_BF16)), b1=b1[:, None].copy(),
        w2T=np.ascontiguousarray(W2.T.astype(NP_BF16)), b2=b2[:, None].copy(),
        conv_wT=conv_wT, conv_b=conv_b, headWT=headWT, head_b=head_b,
    )
    in_maps = []
    for c in range(N_CORES):
        m = dict(common)
        m["xT"] = np.ascontiguousarray(x[c * SHARD:(c + 1) * SHARD, :].T.astype(NP_BF16))
        m["eidx"] = eidx[c]
        m["esw"] = esw[c]
        in_maps.append(m)
    return nc, in_maps


def kernel(x, edge_index, edge_attr,
           W1, b1, W2, b2,
           g1_rel_W, g1_rel_b, g1_root_W,
           g2_rel_W, g2_rel_b, g2_root_W,
           mu_W, mu_b, lv_W, lv_b):
    weights = (W1, b1, W2, b2, g1_rel_W, g1_rel_b, g1_root_W,
               g2_rel_W, g2_rel_b, g2_root_W, mu_W, mu_b, lv_W, lv_b)
    nc, in_maps = _get_compiled(x, edge_index, edge_attr, weights)
    res = bass_utils.run_bass_kernel_spmd(nc, in_maps,
                                          core_ids=list(range(N_CORES)))
    muvT = np.concatenate([res.results[c]["muvT"] for c in range(N_CORES)],
                          axis=1)
    return (np.ascontiguousarray(muvT[:LAT, :].T),
            np.ascontiguousarray(muvT[LAT:, :].T))


# revision 37
# speedup vs baseline: 1.0082x; 1.0082x over previous
"""DRASI encoder (MLP -> GraphConv x2 -> mu/logvar heads) on 8 Trainium2 cores.

Sharding: nodes are split into 8 contiguous shards of 6250. Each core runs the
node-local MLP on its shard (transposed layout, weights as matmul lhsT), the
shards are AllGathered into a full [50000, 128] bf16 feature table in DRAM, and
each core processes the edges whose destination lies in its shard:

  - edges are sorted by dst and bucketed into 96-node "groups"; each
    (group, src-half) bucket is padded to whole 128-edge blocks, with the
    block count unified across cores (max) so all 8 cores share one program;
  - groups are packed into large chunks (<= 96 blocks); one dma_gather per
    (chunk, src-half) fetches source rows from the table (int16 indices, so
    the table is addressed as two 25000-row halves);
  - the selection matrix S_w[e, s] = w_e * (seg_e == s) is precomputed on the
    host in bf16 and streamed in via plain DMA (which does not contend with
    the gather path), so no on-device one-hot build is needed;
  - per-group PE matmuls accumulate aggT = msg.T @ S_w in PSUM and evict to
    an SBUF table in bf16;
  - the GraphConv root-term matmuls run during the AllGather windows into a
    bf16 buffer and are re-injected into the rel-term PSUM accumulation with
    an identity matmul, so the linear phase is one PSUM group + one
    bias+relu activation per 512-column tile, interleaved with aggregation;
  - linear outputs are PE-transposed back to natural layout and published
    with two strided DMAs per AllGather table.

Outputs (mu, logvar) are computed per shard and concatenated on the host.
"""
import sys
sys.path.insert(0, '/opt/trn_rl_repo')

import numpy as np
import concourse.bass as bass
import concourse.bacc as bacc
import concourse.mybir as mybir
from concourse.tile import TileContext
from concourse.masks import make_identity
from concourse import bass_utils

P = 128
N_CORES = 8
N_NODES = 50000
IN_DIM = 512
HID = 128
LAT = 32
SHARD = N_NODES // N_CORES          # 6250
HALF = N_NODES // 2                 # 25000
W = 96                              # nodes per segment group (PSUM tile width)
MAXBLK = 72                         # max 128-edge blocks per chunk
N_GROUPS = (SHARD + W - 1) // W     # 66
N_TILES = [512] * (SHARD // 512) + ([SHARD % 512] if SHARD % 512 else [])
F32 = mybir.dt.float32
BF16 = mybir.dt.bfloat16
I16 = mybir.dt.int16
import ml_dtypes
NP_BF16 = ml_dtypes.bfloat16


# ---------------------------------------------------------------- host prep --

def _unified_structure(per_core_edges):
    """per_core_edges: list of (src, dst_local, w) sorted by dst_local.
    Returns (chunk_meta, per-core idx arrays, per-core S_w arrays)."""
    buckets = [[[None, None] for _ in range(N_GROUPS)] for _ in range(N_CORES)]
    for c, (src, dstl, wgt) in enumerate(per_core_edges):
        grp = dstl // W
        for g in range(N_GROUPS):
            sel = grp == g
            gs, gd, gw = src[sel], dstl[sel], wgt[sel]
            hi = gs >= HALF
            for h in (0, 1):
                m = hi == bool(h)
                buckets[c][g][h] = (gs[m] - h * HALF, gd[m] - g * W, gw[m])

    # unified block count per (group, half): max over cores, >= 1 block per
    # group total so every group gets an eviction
    B = np.zeros((N_GROUPS, 2), np.int64)
    for g in range(N_GROUPS):
        for h in (0, 1):
            B[g, h] = max((buckets[c][g][h][0].shape[0] + P - 1) // P
                          for c in range(N_CORES))
        if B[g, 0] == 0 and B[g, 1] == 0:
            B[g, 0] = 1

    # pack consecutive groups into chunks of <= MAXBLK blocks (lo and hi
    # share one msg tile: lo runs occupy blocks [0, nblk_lo), hi the rest).
    # The first chunks are small so the first segment matmuls start early,
    # and the final chunk is small to shorten the conv tail.
    caps = [24, 48]
    chunks = []
    cur, cur_n = [], 0
    for g in range(N_GROUPS):
        nb = int(B[g, 0] + B[g, 1])
        cap = caps[len(chunks)] if len(chunks) < len(caps) else MAXBLK
        if cur and cur_n + nb > cap:
            chunks.append(cur)
            cur, cur_n = [], 0
        cur.append(g)
        cur_n += nb
    if cur:
        chunks.append(cur)
    # rebalance the tail: if the last chunk is large, split it
    if len(chunks) >= 2:
        last = chunks[-1]
        nlast = sum(int(B[g, 0] + B[g, 1]) for g in last)
        if nlast > 32 and len(last) >= 2:
            csum = 0
            for i in range(len(last) - 1, -1, -1):
                csum += int(B[last[i], 0] + B[last[i], 1])
                if csum >= 16:
                    break
            if i > 0:
                chunks[-1] = last[:i]
                chunks.append(last[i:])

    chunk_meta = []
    core_idx = [[] for _ in range(N_CORES)]
    core_sw = [[] for _ in range(N_CORES)]
    for groups in chunks:
        nblk_lo = int(sum(B[g, 0] for g in groups))
        nblk_hi = int(sum(B[g, 1] for g in groups))
        nblk = nblk_lo + nblk_hi
        runs = []
        b = 0
        for h in (0, 1):
            for g in groups:
                nb = int(B[g, h])
                if nb:
                    runs.append((g, h, b, b + nb))
                    b += nb
        chunk_meta.append(dict(nblk=nblk, nblk_lo=nblk_lo, runs=runs,
                               groups=list(groups)))

        for c in range(N_CORES):
            idx_flat = np.zeros(nblk * P, np.int16)
            seg_flat = np.full(nblk * P, -1.0, np.float32)
            w_flat = np.zeros(nblk * P, np.float32)
            for (g, h, b0, b1_) in runs:
                ids, segs, ws = buckets[c][g][h]
                n = ids.shape[0]
                o = b0 * P
                idx_flat[o:o + n] = ids.astype(np.int16)
                seg_flat[o:o + n] = segs.astype(np.float32)
                w_flat[o:o + n] = ws
            idx_t = np.tile(idx_flat.reshape(nblk * 8, 16).T, (8, 1))
            core_idx[c].append(idx_t)
            # S_w[e, s, b] = w * (seg == s); padded slots have seg=-1 -> zero
            seg_b = seg_flat.reshape(nblk, P)          # [b, e]
            w_b = w_flat.reshape(nblk, P)
            sw = (seg_b[:, :, None] == np.arange(W)[None, None, :])
            sw = sw * w_b[:, :, None]                  # [b, e, s]
            # device tile layout: [e(part), b, s]
            sw_t = np.ascontiguousarray(
                sw.transpose(1, 0, 2).astype(NP_BF16)) # [e, b, s]
            core_sw[c].append(sw_t.reshape(P, W * nblk))

    eidx = [np.ascontiguousarray(np.concatenate(core_idx[c], axis=1))
            for c in range(N_CORES)]
    esw = [np.ascontiguousarray(np.concatenate(core_sw[c], axis=1))
           for c in range(N_CORES)]
    return chunk_meta, eidx, esw


# ------------------------------------------------------------- device build --

def _build(metas, idx_cols, sw_cols):
    nc = bacc.Bacc(None, target_bir_lowering=False, num_devices=N_CORES,
                   num_swdge_queues=2)

    xT = nc.dram_tensor("xT", [IN_DIM, SHARD], BF16, kind="ExternalInput")
    w1T = nc.dram_tensor("w1T", [IN_DIM, HID], BF16, kind="ExternalInput")
    b1 = nc.dram_tensor("b1", [HID, 1], F32, kind="ExternalInput")
    w2T = nc.dram_tensor("w2T", [HID, HID], BF16, kind="ExternalInput")
    b2 = nc.dram_tensor("b2", [HID, 1], F32, kind="ExternalInput")
    conv_wT = nc.dram_tensor("conv_wT", [2, 2, HID, HID], BF16, kind="ExternalInput")
    conv_b = nc.dram_tensor("conv_b", [2, HID, 1], F32, kind="ExternalInput")
    headWT = nc.dram_tensor("headWT", [HID, 2 * LAT], BF16, kind="ExternalInput")
    head_b = nc.dram_tensor("head_b", [2 * LAT, 1], F32, kind="ExternalInput")
    eidx = nc.dram_tensor("eidx", [P, idx_cols], I16, kind="ExternalInput")
    esw = nc.dram_tensor("esw", [P, sw_cols], BF16, kind="ExternalInput")
    muv_out = nc.dram_tensor("muvT", [2 * LAT, SHARD], F32, kind="ExternalOutput")

    ag_in = [nc.dram_tensor(f"ag_in{i}", [SHARD, HID], BF16) for i in range(2)]
    tables = [nc.dram_tensor(f"h_full{i}", [N_NODES, HID], BF16,
                             addr_space="Shared") for i in range(2)]

    NT_FULL = SHARD // P            # 48 full 128-row publish tiles
    TAIL = SHARD - NT_FULL * P      # 106

    with TileContext(nc) as tc:
        with (
            tc.tile_pool(name="const", bufs=1) as cp,
            tc.tile_pool(name="big", bufs=1) as bigp,
            tc.tile_pool(name="work", bufs=3) as wp,
            tc.tile_pool(name="msgp", bufs=2) as msgp,
            tc.tile_pool(name="ps_lin", bufs=3, space="PSUM") as ps_lin,
            tc.tile_pool(name="ps_tr", bufs=2, space="PSUM") as ps_tr,
            tc.tile_pool(name="ps_agg", bufs=3, space="PSUM") as ps_agg,
        ):
            # ---- constants: keep SP free for x loads; weights go on Act
            w1t_sb = [cp.tile([P, HID], BF16, tag=f"w1_{k}", name=f"w1t_{k}") for k in range(4)]
            for k in range(4):
                nc.scalar.dma_start(out=w1t_sb[k][:], in_=w1T[k * P:(k + 1) * P, :])
            b1_sb = cp.tile([P, 1], F32, tag="b1")
            nc.scalar.dma_start(out=b1_sb[:], in_=b1[:, :])
            w2t_sb = cp.tile([P, HID], BF16, tag="w2")
            nc.scalar.dma_start(out=w2t_sb[:], in_=w2T[:, :])
            b2_sb = cp.tile([P, 1], F32, tag="b2")
            nc.scalar.dma_start(out=b2_sb[:], in_=b2[:, :])
            # conv-phase constants are emitted after the first AllGather so
            # their DMAs land in the collective window (see below)
            cw_sb = [[cp.tile([P, HID], BF16, tag=f"cw{l}{m}", name=f"cw_{l}_{m}") for m in range(2)]
                     for l in range(2)]
            cb_sb = [cp.tile([P, 1], F32, tag=f"cb{l}", name=f"cb_{l}") for l in range(2)]
            hw_sb = cp.tile([P, 2 * LAT], BF16, tag="hw")
            hb_sb = cp.tile([2 * LAT, 1], F32, tag="hb")
            ident = cp.tile([P, P], BF16, tag="ident")
            make_identity(nc, ident[:])

            hA = bigp.tile([P, SHARD], BF16, tag="hA")   # h2T, then h4T
            hB = bigp.tile([P, SHARD], BF16, tag="hB")   # h3T
            aggT = bigp.tile([P, SHARD], BF16, tag="aggT")
            rootT_sb = bigp.tile([P, SHARD], BF16, tag="rootT")
            rootT = [rootT_sb, rootT_sb]
            natf = [bigp.tile([P, NT_FULL, HID], BF16, tag=f"natf{i}",
                              name=f"natf_{i}") for i in range(2)]
            natt = [bigp.tile([P, HID], BF16, tag=f"natt{i}",
                              name=f"natt_{i}") for i in range(2)]
            muvT = bigp.tile([2 * LAT, SHARD], F32, tag="muvT")

            def emit_transpose_tiles(hT_tile, t_idx, n0, n1, eng_flip=None):
                # transpose hT[:, n0:n1] into natural-layout staging tiles;
                # PSUM->SBUF eviction on DVE (Act is busy with relus/evicts)
                t = n0 // P
                while n0 < n1:
                    w_ = min(P, n1 - n0)
                    tr_ps = ps_tr.tile([P, P], BF16, space="PSUM", tag="tr",
                                       name="trp")
                    nc.tensor.transpose(out=tr_ps[:w_, :],
                                        in_=hT_tile[:, n0:n0 + w_],
                                        identity=ident[:])
                    dst = natf[t_idx][:, t, :] if w_ == P else natt[t_idx][:TAIL, :]
                    nc.vector.tensor_copy(
                        out=dst[:w_, :] if w_ == P else dst,
                        in_=tr_ps[:w_, :])
                    n0 += w_
                    t += 1

            def emit_publish(t_idx):
                nc.sync.dma_start(
                    out=ag_in[t_idx][:NT_FULL * P, :].rearrange(
                        "(t r) h -> r t h", r=P),
                    in_=natf[t_idx][:, :, :])
                nc.sync.dma_start(out=ag_in[t_idx][NT_FULL * P:, :],
                                  in_=natt[t_idx][:TAIL, :])

            def emit_allgather(t_idx):
                nc.gpsimd.collective_compute(
                    "AllGather", mybir.AluOpType.bypass,
                    replica_groups=[list(range(N_CORES))],
                    ins=[ag_in[t_idx][:, :]],
                    outs=[tables[t_idx][:, :]],
                )

            # ---- MLP (bf16 matmuls, f32 psum) ----
            # software-pipelined so the PE stream has no dependency gaps:
            # stage t emits layer-1 matmuls for tile t, layer-2 for t-1, and
            # publish transposes for t-2
            cols = [sum(N_TILES[:i]) for i in range(len(N_TILES))]
            h1_sbs = {}

            def mlp_l1(t):
                nt, col = N_TILES[t], cols[t]
                # spread the 4 partition chunks over SP(3-in-1) and Pool(1)
                xt3 = wp.tile([P, 3, 512], BF16, tag="xt3")
                nc.sync.dma_start(
                    out=xt3[:, :, :nt],
                    in_=xT[0:3 * P, col:col + nt].rearrange(
                        "(k p) n -> p k n", p=P))
                xt1 = wp.tile([P, 512], BF16, tag="xt1")
                nc.gpsimd.dma_start(out=xt1[:, :nt],
                                    in_=xT[3 * P:4 * P, col:col + nt])
                h1_ps = ps_lin.tile([P, 512], F32, space="PSUM", tag="lin")
                for k in range(4):
                    rhs = xt1[:, :nt] if k == 3 else xt3[:, k, :nt]
                    nc.tensor.matmul(out=h1_ps[:, :nt], lhsT=w1t_sb[k][:],
                                     rhs=rhs,
                                     start=(k == 0), stop=(k == 3))
                h1_sb = wp.tile([P, 512], BF16, tag="h1")
                nc.scalar.activation(out=h1_sb[:, :nt], in_=h1_ps[:, :nt],
                                     func=mybir.ActivationFunctionType.Relu,
                                     bias=b1_sb[:])
                h1_sbs[t] = h1_sb

            def mlp_l2(t):
                nt, col = N_TILES[t], cols[t]
                h2_ps = ps_lin.tile([P, 512], F32, space="PSUM", tag="lin")
                nc.tensor.matmul(out=h2_ps[:, :nt], lhsT=w2t_sb[:],
                                 rhs=h1_sbs.pop(t)[:, :nt],
                                 start=True, stop=True)
                nc.scalar.activation(out=hA[:, col:col + nt], in_=h2_ps[:, :nt],
                                     func=mybir.ActivationFunctionType.Relu,
                                     bias=b2_sb[:])

            NTI = len(N_TILES)
            for t in range(NTI + 2):
                if t < NTI:
                    mlp_l1(t)
                if 1 <= t <= NTI:
                    mlp_l2(t - 1)
                if t >= 2:
                    emit_transpose_tiles(hA, 0, cols[t - 2],
                                         cols[t - 2] + N_TILES[t - 2])
            emit_publish(0)
            emit_allgather(0)

            # conv-phase constant loads land here, inside the AG0 window
            for l in range(2):
                for m in range(2):
                    nc.scalar.dma_start(out=cw_sb[l][m][:], in_=conv_wT[l, m, :, :])
                nc.scalar.dma_start(out=cb_sb[l][:], in_=conv_b[l, :, :])
            nc.scalar.dma_start(out=hw_sb[:], in_=headWT[:, :])
            nc.scalar.dma_start(out=hb_sb[:], in_=head_b[:, :])

            def emit_root(layer, hT_in):
                col = 0
                for nt in N_TILES:
                    ps = ps_lin.tile([P, 512], F32, space="PSUM", tag="lin")
                    nc.tensor.matmul(out=ps[:, :nt], lhsT=cw_sb[layer][1][:],
                                     rhs=hT_in[:, col:col + nt],
                                     start=True, stop=True)
                    nc.scalar.activation(out=rootT[layer][:, col:col + nt],
                                         in_=ps[:, :nt],
                                         func=mybir.ActivationFunctionType.Copy)
                    col += nt

            def conv_layer(layer, hT_in, hT_out, table, pub_idx=None,
                           tile_tail=None):
                # aggregation: chunked gathers + per-group PE segment sums
                icol = 0
                scol = 0
                done_g = 0          # groups fully evicted so far
                done_tiles = 0      # linear tiles emitted so far

                def emit_ready_linear(done_g, done_tiles, force=False):
                    # emit linear tiles whose agg columns are complete
                    avail = min(done_g * W, SHARD)
                    col = done_tiles * 512
                    while done_tiles < len(N_TILES):
                        nt = N_TILES[done_tiles]
                        if col + nt > avail and not force:
                            break
                        ps = ps_lin.tile([P, 512], F32, space="PSUM", tag="lin")
                        nc.tensor.matmul(out=ps[:, :nt], lhsT=cw_sb[layer][0][:],
                                         rhs=aggT[:, col:col + nt],
                                         start=True, stop=False)
                        nc.tensor.matmul(out=ps[:, :nt], lhsT=ident[:],
                                         rhs=rootT[layer][:, col:col + nt],
                                         start=False, stop=True)
                        nc.scalar.activation(
                            out=hT_out[:, col:col + nt], in_=ps[:, :nt],
                            func=mybir.ActivationFunctionType.Relu,
                            bias=cb_sb[layer][:])
                        if pub_idx is not None:
                            emit_transpose_tiles(hT_out, pub_idx, col, col + nt)
                        if tile_tail is not None:
                            tile_tail(col, nt)
                        col += nt
                        done_tiles += 1
                    return done_tiles

                for ci, meta in enumerate(metas):
                    nblk, nblk_lo = meta["nblk"], meta["nblk_lo"]
                    qn = 0
                    idx_t = wp.tile([P, MAXBLK * 8], I16, tag="eidx")
                    nc.scalar.dma_start(out=idx_t[:, :nblk * 8],
                                        in_=eidx[:, icol:icol + nblk * 8])
                    # S_w is a large transfer; DMAs hold the issuing engine,
                    # so it lives on SP (idle during the conv phase)
                    s_w = msgp.tile([P, MAXBLK, W], BF16, tag="sw")
                    nc.sync.dma_start(
                        out=s_w[:, :nblk, :],
                        in_=esw[:, scol:scol + W * nblk].rearrange(
                            "p (b s) -> p b s", s=W))

                    msg = msgp.tile([P, MAXBLK, HID], BF16, tag="msg")
                    if nblk_lo:
                        nc.gpsimd.dma_gather(
                            out_ap=msg[:, :nblk_lo, :], in_ap=table[:HALF, :],
                            idxs_ap=idx_t[:, :nblk_lo * 8],
                            num_idxs=nblk_lo * P, num_idxs_reg=nblk_lo * P,
                            elem_size=HID, single_packet=False,
                            queue_num=qn)
                    if nblk - nblk_lo:
                        nh = nblk - nblk_lo
                        nc.gpsimd.dma_gather(
                            out_ap=msg[:, nblk_lo:nblk, :], in_ap=table[HALF:, :],
                            idxs_ap=idx_t[:, nblk_lo * 8:nblk * 8],
                            num_idxs=nh * P, num_idxs_reg=nh * P,
                            elem_size=HID, single_packet=False,
                            queue_num=qn)

                    # one psum + one eviction per group: a group's lo and hi
                    # runs accumulate into the same tile
                    by_group = {}
                    for (g, h, b0, b1_) in meta["runs"]:
                        by_group.setdefault(g, []).append((h, b0, b1_))
                    for g in meta["groups"]:
                        ps = ps_agg.tile([P, W], F32, space="PSUM", tag="agg")
                        blocks = [(h, b) for (h, b0, b1_) in by_group[g]
                                  for b in range(b0, b1_)]
                        for i, (h, b) in enumerate(blocks):
                            nc.tensor.matmul(out=ps[:], lhsT=msg[:, b, :],
                                             rhs=s_w[:, b, :],
                                             start=(i == 0),
                                             stop=(i == len(blocks) - 1))
                        gw = min(W, SHARD - g * W)
                        nc.scalar.activation(
                            out=aggT[:, g * W:g * W + gw], in_=ps[:, :gw],
                            func=mybir.ActivationFunctionType.Copy)
                    icol += nblk * 8
                    scol += W * nblk
                    done_g += len(meta["groups"])
                    done_tiles = emit_ready_linear(done_g, done_tiles)
                done_tiles = emit_ready_linear(done_g, done_tiles, force=True)
                if pub_idx is not None:
                    emit_publish(pub_idx)

            emit_root(0, hA)
            conv_layer(0, hA, hB, tables[0], pub_idx=1)
            emit_allgather(1)
            emit_root(1, hB)

            # ---- heads fused into conv2's linear phase ----
            def head_tail(col, nt):
                ps = ps_lin.tile([2 * LAT, 512], F32, space="PSUM", tag="lin",
                                 name="headps")
                nc.tensor.matmul(out=ps[:, :nt], lhsT=hw_sb[:],
                                 rhs=hA[:, col:col + nt], start=True, stop=True)
                nc.vector.tensor_tensor(
                    out=muvT[:, col:col + nt], in0=ps[:, :nt],
                    in1=hb_sb[:].to_broadcast([2 * LAT, nt]),
                    op=mybir.AluOpType.add)

            conv_layer(1, hB, hA, tables[1], tile_tail=head_tail)
            HS = SHARD // 2
            nc.sync.dma_start(out=muv_out[:, :HS], in_=muvT[:, :HS])
            nc.scalar.dma_start(out=muv_out[:, HS:], in_=muvT[:, HS:])

    nc.finalize()
    return nc


# -------------------------------------------------------------------- driver --

def _get_compiled(x, edge_index, edge_attr, weights):
    src = np.asarray(edge_index[0]).astype(np.int64)
    dst = np.asarray(edge_index[1]).astype(np.int64)
    wgt = np.asarray(edge_attr, dtype=np.float32)
    x = np.asarray(x, dtype=np.float32)

    per_core_edges = []
    for c in range(N_CORES):
        sel = (dst >= c * SHARD) & (dst < (c + 1) * SHARD)
        s, d, wv = src[sel], dst[sel] - c * SHARD, wgt[sel]
        order = np.argsort(d, kind="stable")
        per_core_edges.append((s[order], d[order], wv[order]))

    metas, eidx, esw = _unified_structure(per_core_edges)
    idx_cols = sum(m["nblk"] * 8 for m in metas)
    sw_cols = sum(m["nblk"] * W for m in metas)

    nc = _build(metas, idx_cols, sw_cols)

    (W1, b1, W2, b2, g1_rel_W, g1_rel_b, g1_root_W,
     g2_rel_W, g2_rel_b, g2_root_W, mu_W, mu_b, lv_W, lv_b) = [
        np.asarray(w, dtype=np.float32) for w in weights]

    conv_wT = np.stack([
        np.stack([g1_rel_W.T, g1_root_W.T]),
        np.stack([g2_rel_W.T, g2_root_W.T]),
    ]).astype(NP_BF16).copy()
    conv_b = np.stack([g1_rel_b[:, None], g2_rel_b[:, None]]).copy()
    headWT = np.ascontiguousarray(
        np.concatenate([mu_W, lv_W], axis=0).T.astype(NP_BF16))
    head_b = np.concatenate([mu_b, lv_b])[:, None].copy()

    common = dict(
        w1T=np.ascontiguousarray(W1.T.astype(NP_BF16)), b1=b1[:, None].copy(),
        w2T=np.ascontiguousarray(W2.T.astype(NP_BF16)), b2=b2[:, None].copy(),
        conv_wT=conv_wT, conv_b=conv_b, headWT=headWT, head_b=head_b,
    )
    in_maps = []
    for c in range(N_CORES):
        m = dict(common)
        m["xT"] = np.ascontiguousarray(x[c * SHARD:(c + 1) * SHARD, :].T.astype(NP_BF16))
        m["eidx"] = eidx[c]
        m["esw"] = esw[c]
        in_maps.append(m)
    return nc, in_maps


def kernel(x, edge_index, edge_attr,
           W1, b1, W2, b2,
           g1_rel_W, g1_rel_b, g1_root_W,
           g2_rel_W, g2_rel_b, g2_root_W,
           mu_W, mu_b, lv_W, lv_b):
    weights = (W1, b1, W2, b2, g1_rel_W, g1_rel_b, g1_root_W,
               g2_rel_W, g2_rel_b, g2_root_W, mu_W, mu_b, lv_W, lv_b)
    nc, in_maps = _get_compiled(x, edge_index, edge_attr, weights)
    res = bass_utils.run_bass_kernel_spmd(nc, in_maps,
                                          core_ids=list(range(N_CORES)))
    muvT = np.concatenate([res.results[c]["muvT"] for c in range(N_CORES)],
                          axis=1)
    return (np.ascontiguousarray(muvT[:LAT, :].T),
            np.ascontiguousarray(muvT[LAT:, :].T))


# revision 44
# speedup vs baseline: 1.0162x; 1.0080x over previous
"""DRASI encoder (MLP -> GraphConv x2 -> mu/logvar heads) on 8 Trainium2 cores.

Sharding: nodes are split into 8 contiguous shards of 6250. Each core runs the
node-local MLP on its shard (transposed layout, weights as matmul lhsT), the
shards are AllGathered into a full [50000, 128] bf16 feature table in DRAM, and
each core processes the edges whose destination lies in its shard:

  - edges are sorted by dst and bucketed into 96-node "groups"; each
    (group, src-half) bucket is padded to whole 128-edge blocks, with the
    block count unified across cores (max) so all 8 cores share one program;
  - groups are packed into large chunks (<= 96 blocks); one dma_gather per
    (chunk, src-half) fetches source rows from the table (int16 indices, so
    the table is addressed as two 25000-row halves);
  - the selection matrix S_w[e, s] = w_e * (seg_e == s) is precomputed on the
    host in bf16 and streamed in via plain DMA (which does not contend with
    the gather path), so no on-device one-hot build is needed;
  - per-group PE matmuls accumulate aggT = msg.T @ S_w in PSUM and evict to
    an SBUF table in bf16;
  - the GraphConv root-term matmuls run during the AllGather windows into a
    bf16 buffer and are re-injected into the rel-term PSUM accumulation with
    an identity matmul, so the linear phase is one PSUM group + one
    bias+relu activation per 512-column tile, interleaved with aggregation;
  - linear outputs are PE-transposed back to natural layout and published
    with two strided DMAs per AllGather table.

Outputs (mu, logvar) are computed per shard and concatenated on the host.
"""
import sys
sys.path.insert(0, '/opt/trn_rl_repo')

import numpy as np
import concourse.bass as bass
import concourse.bacc as bacc
import concourse.mybir as mybir
from concourse.tile import TileContext
from concourse.masks import make_identity
from concourse import bass_utils

P = 128
N_CORES = 8
N_NODES = 50000
IN_DIM = 512
HID = 128
LAT = 32
SHARD = N_NODES // N_CORES          # 6250
HALF = N_NODES // 2                 # 25000
W = 96                              # nodes per segment group (PSUM tile width)
MAXBLK = 72                         # max 128-edge blocks per chunk
N_GROUPS = (SHARD + W - 1) // W     # 66
N_TILES = [512] * (SHARD // 512) + ([SHARD % 512] if SHARD % 512 else [])
F32 = mybir.dt.float32
BF16 = mybir.dt.bfloat16
I16 = mybir.dt.int16
import ml_dtypes
NP_BF16 = ml_dtypes.bfloat16


# ---------------------------------------------------------------- host prep --

def _unified_structure(per_core_edges):
    """per_core_edges: list of (src, dst_local, w) sorted by dst_local.
    Returns (chunk_meta, per-core idx arrays, per-core S_w arrays)."""
    buckets = [[[None, None] for _ in range(N_GROUPS)] for _ in range(N_CORES)]
    for c, (src, dstl, wgt) in enumerate(per_core_edges):
        grp = dstl // W
        for g in range(N_GROUPS):
            sel = grp == g
            gs, gd, gw = src[sel], dstl[sel], wgt[sel]
            hi = gs >= HALF
            for h in (0, 1):
                m = hi == bool(h)
                buckets[c][g][h] = (gs[m] - h * HALF, gd[m] - g * W, gw[m])

    # unified block count per (group, half): max over cores, >= 1 block per
    # group total so every group gets an eviction
    B = np.zeros((N_GROUPS, 2), np.int64)
    for g in range(N_GROUPS):
        for h in (0, 1):
            B[g, h] = max((buckets[c][g][h][0].shape[0] + P - 1) // P
                          for c in range(N_CORES))
        if B[g, 0] == 0 and B[g, 1] == 0:
            B[g, 0] = 1

    # pack consecutive groups into chunks of <= MAXBLK blocks (lo and hi
    # share one msg tile: lo runs occupy blocks [0, nblk_lo), hi the rest).
    # The first chunks are small so the first segment matmuls start early,
    # and the final chunk is small to shorten the conv tail.
    caps = [24, 48]
    chunks = []
    cur, cur_n = [], 0
    for g in range(N_GROUPS):
        nb = int(B[g, 0] + B[g, 1])
        cap = caps[len(chunks)] if len(chunks) < len(caps) else MAXBLK
        if cur and cur_n + nb > cap:
            chunks.append(cur)
            cur, cur_n = [], 0
        cur.append(g)
        cur_n += nb
    if cur:
        chunks.append(cur)
    # rebalance the tail: if the last chunk is large, split it
    if len(chunks) >= 2:
        last = chunks[-1]
        nlast = sum(int(B[g, 0] + B[g, 1]) for g in last)
        if nlast > 32 and len(last) >= 2:
            csum = 0
            for i in range(len(last) - 1, -1, -1):
                csum += int(B[last[i], 0] + B[last[i], 1])
                if csum >= 16:
                    break
            if i > 0:
                chunks[-1] = last[:i]
                chunks.append(last[i:])

    chunk_meta = []
    core_idx = [[] for _ in range(N_CORES)]
    core_sw = [[] for _ in range(N_CORES)]
    for groups in chunks:
        nblk_lo = int(sum(B[g, 0] for g in groups))
        nblk_hi = int(sum(B[g, 1] for g in groups))
        nblk = nblk_lo + nblk_hi
        runs = []
        b = 0
        for h in (0, 1):
            for g in groups:
                nb = int(B[g, h])
                if nb:
                    runs.append((g, h, b, b + nb))
                    b += nb
        chunk_meta.append(dict(nblk=nblk, nblk_lo=nblk_lo, runs=runs,
                               groups=list(groups)))

        for c in range(N_CORES):
            idx_flat = np.zeros(nblk * P, np.int16)
            seg_flat = np.full(nblk * P, -1.0, np.float32)
            w_flat = np.zeros(nblk * P, np.float32)
            for (g, h, b0, b1_) in runs:
                ids, segs, ws = buckets[c][g][h]
                n = ids.shape[0]
                o = b0 * P
                idx_flat[o:o + n] = ids.astype(np.int16)
                seg_flat[o:o + n] = segs.astype(np.float32)
                w_flat[o:o + n] = ws
            idx_t = np.tile(idx_flat.reshape(nblk * 8, 16).T, (8, 1))
            core_idx[c].append(idx_t)
            # S_w[e, s, b] = w * (seg == s); padded slots have seg=-1 -> zero
            seg_b = seg_flat.reshape(nblk, P)          # [b, e]
            w_b = w_flat.reshape(nblk, P)
            sw = (seg_b[:, :, None] == np.arange(W)[None, None, :])
            sw = sw * w_b[:, :, None]                  # [b, e, s]
            # device tile layout: [e(part), b, s]
            sw_t = np.ascontiguousarray(
                sw.transpose(1, 0, 2).astype(NP_BF16)) # [e, b, s]
            core_sw[c].append(sw_t.reshape(P, W * nblk))

    eidx = [np.ascontiguousarray(np.concatenate(core_idx[c], axis=1))
            for c in range(N_CORES)]
    esw = [np.ascontiguousarray(np.concatenate(core_sw[c], axis=1))
           for c in range(N_CORES)]
    return chunk_meta, eidx, esw


# ------------------------------------------------------------- device build --

def _build(metas, idx_cols, sw_cols):
    nc = bacc.Bacc(None, target_bir_lowering=False, num_devices=N_CORES,
                   num_swdge_queues=2)

    xT = nc.dram_tensor("xT", [IN_DIM, SHARD], BF16, kind="ExternalInput")
    w1T = nc.dram_tensor("w1T", [IN_DIM, HID], BF16, kind="ExternalInput")
    b1 = nc.dram_tensor("b1", [HID, 1], F32, kind="ExternalInput")
    w2T = nc.dram_tensor("w2T", [HID, HID], BF16, kind="ExternalInput")
    b2 = nc.dram_tensor("b2", [HID, 1], F32, kind="ExternalInput")
    conv_wT = nc.dram_tensor("conv_wT", [2, 2, HID, HID], BF16, kind="ExternalInput")
    conv_b = nc.dram_tensor("conv_b", [2, HID, 1], F32, kind="ExternalInput")
    headWT = nc.dram_tensor("headWT", [HID, 2 * LAT], BF16, kind="ExternalInput")
    head_b = nc.dram_tensor("head_b", [2 * LAT, 1], F32, kind="ExternalInput")
    eidx = nc.dram_tensor("eidx", [P, idx_cols], I16, kind="ExternalInput")
    esw = nc.dram_tensor("esw", [P, sw_cols], BF16, kind="ExternalInput")
    muv_out = nc.dram_tensor("muvT", [2 * LAT, SHARD], F32, kind="ExternalOutput")

    ag_in = [nc.dram_tensor(f"ag_in{i}", [SHARD, HID], BF16) for i in range(2)]
    tables = [nc.dram_tensor(f"h_full{i}", [N_NODES, HID], BF16,
                             addr_space="Shared") for i in range(2)]

    NT_FULL = SHARD // P            # 48 full 128-row publish tiles
    TAIL = SHARD - NT_FULL * P      # 106

    with TileContext(nc) as tc:
        with (
            tc.tile_pool(name="const", bufs=1) as cp,
            tc.tile_pool(name="big", bufs=1) as bigp,
            tc.tile_pool(name="work", bufs=3) as wp,
            tc.tile_pool(name="msgp", bufs=2) as msgp,
            tc.tile_pool(name="ps_tr", bufs=2, space="PSUM") as ps_tr,
        ):
            # ---- constants: keep SP free for x loads; weights go on Act
            w1t_sb = [cp.tile([P, HID], BF16, tag=f"w1_{k}", name=f"w1t_{k}") for k in range(4)]
            for k in range(4):
                nc.scalar.dma_start(out=w1t_sb[k][:], in_=w1T[k * P:(k + 1) * P, :])
            b1_sb = cp.tile([P, 1], F32, tag="b1")
            nc.scalar.dma_start(out=b1_sb[:], in_=b1[:, :])
            w2t_sb = cp.tile([P, HID], BF16, tag="w2")
            nc.scalar.dma_start(out=w2t_sb[:], in_=w2T[:, :])
            b2_sb = cp.tile([P, 1], F32, tag="b2")
            nc.scalar.dma_start(out=b2_sb[:], in_=b2[:, :])
            # conv-phase constants are emitted after the first AllGather so
            # their DMAs land in the collective window (see below)
            cw_sb = [[cp.tile([P, HID], BF16, tag=f"cw{l}{m}", name=f"cw_{l}_{m}") for m in range(2)]
                     for l in range(2)]
            cb_sb = [cp.tile([P, 1], F32, tag=f"cb{l}", name=f"cb_{l}") for l in range(2)]
            hw_sb = cp.tile([P, 2 * LAT], BF16, tag="hw")
            hb_sb = cp.tile([2 * LAT, 1], F32, tag="hb")
            ident = cp.tile([P, P], BF16, tag="ident")
            make_identity(nc, ident[:])

            hA = bigp.tile([P, SHARD], BF16, tag="hA")   # h2T, then h4T
            hB = bigp.tile([P, SHARD], BF16, tag="hB")   # h3T
            aggT = bigp.tile([P, SHARD], BF16, tag="aggT")
            rootT_sb = bigp.tile([P, SHARD], BF16, tag="rootT")
            rootT = [rootT_sb, rootT_sb]
            natf = [bigp.tile([P, NT_FULL, HID], BF16, tag=f"natf{i}",
                              name=f"natf_{i}") for i in range(2)]
            natt = [bigp.tile([P, HID], BF16, tag=f"natt{i}",
                              name=f"natt_{i}") for i in range(2)]
            muvT = bigp.tile([2 * LAT, SHARD], F32, tag="muvT")

            def emit_transpose_tiles(hT_tile, t_idx, n0, n1, eng_flip=None):
                # transpose hT[:, n0:n1] into natural-layout staging tiles;
                # PSUM->SBUF eviction on DVE (Act is busy with relus/evicts)
                t = n0 // P
                while n0 < n1:
                    w_ = min(P, n1 - n0)
                    tr_ps = ps_tr.tile([P, P], BF16, space="PSUM", tag="tr",
                                       name="trp")
                    nc.tensor.transpose(out=tr_ps[:w_, :],
                                        in_=hT_tile[:, n0:n0 + w_],
                                        identity=ident[:])
                    dst = natf[t_idx][:, t, :] if w_ == P else natt[t_idx][:TAIL, :]
                    nc.vector.tensor_copy(
                        out=dst[:w_, :] if w_ == P else dst,
                        in_=tr_ps[:w_, :])
                    n0 += w_
                    t += 1

            def emit_publish(t_idx):
                nc.sync.dma_start(
                    out=ag_in[t_idx][:NT_FULL * P, :].rearrange(
                        "(t r) h -> r t h", r=P),
                    in_=natf[t_idx][:, :, :])
                nc.sync.dma_start(out=ag_in[t_idx][NT_FULL * P:, :],
                                  in_=natt[t_idx][:TAIL, :])

            def emit_allgather(t_idx):
                nc.gpsimd.collective_compute(
                    "AllGather", mybir.AluOpType.bypass,
                    replica_groups=[list(range(N_CORES))],
                    ins=[ag_in[t_idx][:, :]],
                    outs=[tables[t_idx][:, :]],
                )

            # ---- MLP (bf16 matmuls, f32 psum) ----
            # software-pipelined so the PE stream has no dependency gaps:
            # stage t emits layer-1 matmuls for tile t, layer-2 for t-1, and
            # publish transposes for t-2
            cols = [sum(N_TILES[:i]) for i in range(len(N_TILES))]
            h1_sbs = {}
            # dedicated MLP PSUM pool (4 banks) scoped to the MLP so the
            # conv-phase pools can use the banks afterwards
            mlp_ps_cm = tc.tile_pool(name="mlp_ps", bufs=4, space="PSUM")
            mlp_ps = mlp_ps_cm.__enter__()

            def mlp_l1(t):
                nt, col = N_TILES[t], cols[t]
                # spread the 4 partition chunks over SP(3-in-1) and Pool(1)
                xt3 = wp.tile([P, 3, 512], BF16, tag="xt3")
                nc.sync.dma_start(
                    out=xt3[:, :, :nt],
                    in_=xT[0:3 * P, col:col + nt].rearrange(
                        "(k p) n -> p k n", p=P))
                xt1 = wp.tile([P, 512], BF16, tag="xt1")
                nc.gpsimd.dma_start(out=xt1[:, :nt],
                                    in_=xT[3 * P:4 * P, col:col + nt])
                h1_ps = mlp_ps.tile([P, 512], F32, space="PSUM", tag="lin")
                for k in range(4):
                    rhs = xt1[:, :nt] if k == 3 else xt3[:, k, :nt]
                    nc.tensor.matmul(out=h1_ps[:, :nt], lhsT=w1t_sb[k][:],
                                     rhs=rhs,
                                     start=(k == 0), stop=(k == 3))
                h1_sb = wp.tile([P, 512], BF16, tag="h1")
                nc.scalar.activation(out=h1_sb[:, :nt], in_=h1_ps[:, :nt],
                                     func=mybir.ActivationFunctionType.Relu,
                                     bias=b1_sb[:])
                h1_sbs[t] = h1_sb

            def mlp_l2(t):
                nt, col = N_TILES[t], cols[t]
                h2_ps = mlp_ps.tile([P, 512], F32, space="PSUM", tag="lin")
                nc.tensor.matmul(out=h2_ps[:, :nt], lhsT=w2t_sb[:],
                                 rhs=h1_sbs.pop(t)[:, :nt],
                                 start=True, stop=True)
                nc.scalar.activation(out=hA[:, col:col + nt], in_=h2_ps[:, :nt],
                                     func=mybir.ActivationFunctionType.Relu,
                                     bias=b2_sb[:])

            NTI = len(N_TILES)
            for t in range(NTI + 2):
                if t < NTI:
                    mlp_l1(t)
                if 1 <= t <= NTI:
                    mlp_l2(t - 1)
                if t >= 2:
                    emit_transpose_tiles(hA, 0, cols[t - 2],
                                         cols[t - 2] + N_TILES[t - 2])
            emit_publish(0)
            mlp_ps_cm.__exit__(None, None, None)
            ps_lin_cm = tc.tile_pool(name="ps_lin", bufs=3, space="PSUM")
            ps_lin = ps_lin_cm.__enter__()
            ps_agg_cm = tc.tile_pool(name="ps_agg", bufs=3, space="PSUM")
            ps_agg = ps_agg_cm.__enter__()
            emit_allgather(0)

            # conv-phase constant loads land here, inside the AG0 window
            for l in range(2):
                for m in range(2):
                    nc.scalar.dma_start(out=cw_sb[l][m][:], in_=conv_wT[l, m, :, :])
                nc.scalar.dma_start(out=cb_sb[l][:], in_=conv_b[l, :, :])
            nc.scalar.dma_start(out=hw_sb[:], in_=headWT[:, :])
            nc.scalar.dma_start(out=hb_sb[:], in_=head_b[:, :])

            def emit_root(layer, hT_in):
                col = 0
                for nt in N_TILES:
                    ps = ps_lin.tile([P, 512], F32, space="PSUM", tag="lin")
                    nc.tensor.matmul(out=ps[:, :nt], lhsT=cw_sb[layer][1][:],
                                     rhs=hT_in[:, col:col + nt],
                                     start=True, stop=True)
                    nc.scalar.activation(out=rootT[layer][:, col:col + nt],
                                         in_=ps[:, :nt],
                                         func=mybir.ActivationFunctionType.Copy)
                    col += nt

            def conv_layer(layer, hT_in, hT_out, table, pub_idx=None,
                           tile_tail=None):
                # aggregation: chunked gathers + per-group PE segment sums
                icol = 0
                scol = 0
                done_g = 0          # groups fully evicted so far
                done_tiles = 0      # linear tiles emitted so far

                def emit_ready_linear(done_g, done_tiles, force=False):
                    # emit linear tiles whose agg columns are complete
                    avail = min(done_g * W, SHARD)
                    col = done_tiles * 512
                    while done_tiles < len(N_TILES):
                        nt = N_TILES[done_tiles]
                        if col + nt > avail and not force:
                            break
                        ps = ps_lin.tile([P, 512], F32, space="PSUM", tag="lin")
                        nc.tensor.matmul(out=ps[:, :nt], lhsT=cw_sb[layer][0][:],
                                         rhs=aggT[:, col:col + nt],
                                         start=True, stop=False)
                        nc.tensor.matmul(out=ps[:, :nt], lhsT=ident[:],
                                         rhs=rootT[layer][:, col:col + nt],
                                         start=False, stop=True)
                        nc.scalar.activation(
                            out=hT_out[:, col:col + nt], in_=ps[:, :nt],
                            func=mybir.ActivationFunctionType.Relu,
                            bias=cb_sb[layer][:])
                        if pub_idx is not None:
                            emit_transpose_tiles(hT_out, pub_idx, col, col + nt)
                        if tile_tail is not None:
                            tile_tail(col, nt)
                        col += nt
                        done_tiles += 1
                    return done_tiles

                for ci, meta in enumerate(metas):
                    nblk, nblk_lo = meta["nblk"], meta["nblk_lo"]
                    qn = 0
                    idx_t = wp.tile([P, MAXBLK * 8], I16, tag="eidx")
                    nc.scalar.dma_start(out=idx_t[:, :nblk * 8],
                                        in_=eidx[:, icol:icol + nblk * 8])
                    # S_w is a large transfer; DMAs hold the issuing engine,
                    # so it lives on SP (idle during the conv phase)
                    s_w = msgp.tile([P, MAXBLK, W], BF16, tag="sw")
                    nc.sync.dma_start(
                        out=s_w[:, :nblk, :],
                        in_=esw[:, scol:scol + W * nblk].rearrange(
                            "p (b s) -> p b s", s=W))

                    msg = msgp.tile([P, MAXBLK, HID], BF16, tag="msg")
                    if nblk_lo:
                        nc.gpsimd.dma_gather(
                            out_ap=msg[:, :nblk_lo, :], in_ap=table[:HALF, :],
                            idxs_ap=idx_t[:, :nblk_lo * 8],
                            num_idxs=nblk_lo * P, num_idxs_reg=nblk_lo * P,
                            elem_size=HID, single_packet=False,
                            queue_num=qn)
                    if nblk - nblk_lo:
                        nh = nblk - nblk_lo
                        nc.gpsimd.dma_gather(
                            out_ap=msg[:, nblk_lo:nblk, :], in_ap=table[HALF:, :],
                            idxs_ap=idx_t[:, nblk_lo * 8:nblk * 8],
                            num_idxs=nh * P, num_idxs_reg=nh * P,
                            elem_size=HID, single_packet=False,
                            queue_num=qn)

                    # one psum + one eviction per group: a group's lo and hi
                    # runs accumulate into the same tile
                    by_group = {}
                    for (g, h, b0, b1_) in meta["runs"]:
                        by_group.setdefault(g, []).append((h, b0, b1_))
                    for g in meta["groups"]:
                        ps = ps_agg.tile([P, W], F32, space="PSUM", tag="agg")
                        blocks = [(h, b) for (h, b0, b1_) in by_group[g]
                                  for b in range(b0, b1_)]
                        for i, (h, b) in enumerate(blocks):
                            nc.tensor.matmul(out=ps[:], lhsT=msg[:, b, :],
                                             rhs=s_w[:, b, :],
                                             start=(i == 0),
                                             stop=(i == len(blocks) - 1))
                        gw = min(W, SHARD - g * W)
                        nc.scalar.activation(
                            out=aggT[:, g * W:g * W + gw], in_=ps[:, :gw],
                            func=mybir.ActivationFunctionType.Copy)
                    icol += nblk * 8
                    scol += W * nblk
                    done_g += len(meta["groups"])
                    done_tiles = emit_ready_linear(done_g, done_tiles)
                done_tiles = emit_ready_linear(done_g, done_tiles, force=True)
                if pub_idx is not None:
                    emit_publish(pub_idx)

            emit_root(0, hA)
            conv_layer(0, hA, hB, tables[0], pub_idx=1)
            emit_allgather(1)
            emit_root(1, hB)

            # ---- heads fused into conv2's linear phase ----
            def head_tail(col, nt):
                ps = ps_lin.tile([2 * LAT, 512], F32, space="PSUM", tag="lin",
                                 name="headps")
                nc.tensor.matmul(out=ps[:, :nt], lhsT=hw_sb[:],
                                 rhs=hA[:, col:col + nt], start=True, stop=True)
                nc.vector.tensor_tensor(
                    out=muvT[:, col:col + nt], in0=ps[:, :nt],
                    in1=hb_sb[:].to_broadcast([2 * LAT, nt]),
                    op=mybir.AluOpType.add)
                # stream the output tile-by-tile so there is no DMA tail
                eng = nc.sync if (col // 512) % 2 == 0 else nc.scalar
                eng.dma_start(out=muv_out[:, col:col + nt],
                              in_=muvT[:, col:col + nt])

            conv_layer(1, hB, hA, tables[1], tile_tail=head_tail)
            ps_agg_cm.__exit__(None, None, None)
            ps_lin_cm.__exit__(None, None, None)

    nc.finalize()
    return nc


# -------------------------------------------------------------------- driver --

def _get_compiled(x, edge_index, edge_attr, weights):
    src = np.asarray(edge_index[0]).astype(np.int64)
    dst = np.asarray(edge_index[1]).astype(np.int64)
    wgt = np.asarray(edge_attr, dtype=np.float32)
    x = np.asarray(x, dtype=np.float32)

    per_core_edges = []
    for c in range(N_CORES):
        sel = (dst >= c * SHARD) & (dst < (c + 1) * SHARD)
        s, d, wv = src[sel], dst[sel] - c * SHARD, wgt[sel]
        order = np.argsort(d, kind="stable")
        per_core_edges.append((s[order], d[order], wv[order]))

    metas, eidx, esw = _unified_structure(per_core_edges)
    idx_cols = sum(m["nblk"] * 8 for m in metas)
    sw_cols = sum(m["nblk"] * W for m in metas)

    nc = _build(metas, idx_cols, sw_cols)

    (W1, b1, W2, b2, g1_rel_W, g1_rel_b, g1_root_W,
     g2_rel_W, g2_rel_b, g2_root_W, mu_W, mu_b, lv_W, lv_b) = [
        np.asarray(w, dtype=np.float32) for w in weights]

    conv_wT = np.stack([
        np.stack([g1_rel_W.T, g1_root_W.T]),
        np.stack([g2_rel_W.T, g2_root_W.T]),
    ]).astype(NP_BF16).copy()
    conv_b = np.stack([g1_rel_b[:, None], g2_rel_b[:, None]]).copy()
    headWT = np.ascontiguousarray(
        np.concatenate([mu_W, lv_W], axis=0).T.astype(NP_BF16))
    head_b = np.concatenate([mu_b, lv_b])[:, None].copy()

    common = dict(
        w1T=np.ascontiguousarray(W1.T.astype(NP_BF16)), b1=b1[:, None].copy(),
        w2T=np.ascontiguousarray(W2.T.astype(NP_BF16)), b2=b2[:, None].copy(),
        conv_wT=conv_wT, conv_b=conv_b, headWT=headWT, head_b=head_b,
    )
    in_maps = []
    for c in range(N_CORES):
        m = dict(common)
        m["xT"] = np.ascontiguousarray(x[c * SHARD:(c + 1) * SHARD, :].T.astype(NP_BF16))
        m["eidx"] = eidx[c]
        m["esw"] = esw[c]
        in_maps.append(m)
    return nc, in_maps


def kernel(x, edge_index, edge_attr,
           W1, b1, W2, b2,
           g1_rel_W, g1_rel_b, g1_root_W,
           g2_rel_W, g2_rel_b, g2_root_W,
           mu_W, mu_b, lv_W, lv_b):
    weights = (W1, b1, W2, b2, g1_rel_W, g1_rel_b, g1_root_W,
               g2_rel_W, g2_rel_b, g2_root_W, mu_W, mu_b, lv_W, lv_b)
    nc, in_maps = _get_compiled(x, edge_index, edge_attr, weights)
    res = bass_utils.run_bass_kernel_spmd(nc, in_maps,
                                          core_ids=list(range(N_CORES)))
    muvT = np.concatenate([res.results[c]["muvT"] for c in range(N_CORES)],
                          axis=1)
    return (np.ascontiguousarray(muvT[:LAT, :].T),
            np.ascontiguousarray(muvT[LAT:, :].T))


# revision 50
# speedup vs baseline: 1.0283x; 1.0119x over previous
"""DRASI encoder (MLP -> GraphConv x2 -> mu/logvar heads) on 8 Trainium2 cores.

Sharding: nodes are split into 8 contiguous shards of 6250. Each core runs the
node-local MLP on its shard (transposed layout, weights as matmul lhsT), the
shards are AllGathered into a full [50000, 128] bf16 feature table in DRAM, and
each core processes the edges whose destination lies in its shard:

  - edges are sorted by dst and bucketed into 96-node "groups"; each
    (group, src-half) bucket is padded to whole 128-edge blocks, with the
    block count unified across cores (max) so all 8 cores share one program;
  - groups are packed into large chunks (<= 96 blocks); one dma_gather per
    (chunk, src-half) fetches source rows from the table (int16 indices, so
    the table is addressed as two 25000-row halves);
  - the selection matrix S_w[e, s] = w_e * (seg_e == s) is precomputed on the
    host in bf16 and streamed in via plain DMA (which does not contend with
    the gather path), so no on-device one-hot build is needed;
  - per-group PE matmuls accumulate aggT = msg.T @ S_w in PSUM and evict to
    an SBUF table in bf16;
  - the GraphConv root-term matmuls run during the AllGather windows into a
    bf16 buffer and are re-injected into the rel-term PSUM accumulation with
    an identity matmul, so the linear phase is one PSUM group + one
    bias+relu activation per 512-column tile, interleaved with aggregation;
  - linear outputs are PE-transposed back to natural layout and published
    with two strided DMAs per AllGather table.

Outputs (mu, logvar) are computed per shard and concatenated on the host.
"""
import sys
sys.path.insert(0, '/opt/trn_rl_repo')

import numpy as np
import concourse.bass as bass
import concourse.bacc as bacc
import concourse.mybir as mybir
from concourse.tile import TileContext
from concourse.masks import make_identity
from concourse import bass_utils

P = 128
N_CORES = 8
N_NODES = 50000
IN_DIM = 512
HID = 128
LAT = 32
SHARD = N_NODES // N_CORES          # 6250
HALF = N_NODES // 2                 # 25000
W = 96                              # nodes per segment group (PSUM tile width)
MAXBLK = 72                         # max 128-edge blocks per chunk
N_GROUPS = (SHARD + W - 1) // W     # 66
N_TILES = [512] * (SHARD // 512) + ([SHARD % 512] if SHARD % 512 else [])
F32 = mybir.dt.float32
BF16 = mybir.dt.bfloat16
I16 = mybir.dt.int16
import ml_dtypes
NP_BF16 = ml_dtypes.bfloat16


# ---------------------------------------------------------------- host prep --

def _unified_structure(per_core_edges):
    """per_core_edges: list of (src, dst_local, w) sorted by dst_local.
    Returns (chunk_meta, per-core idx arrays, per-core S_w arrays)."""
    buckets = [[[None, None] for _ in range(N_GROUPS)] for _ in range(N_CORES)]
    for c, (src, dstl, wgt) in enumerate(per_core_edges):
        grp = dstl // W
        for g in range(N_GROUPS):
            sel = grp == g
            gs, gd, gw = src[sel], dstl[sel], wgt[sel]
            hi = gs >= HALF
            for h in (0, 1):
                m = hi == bool(h)
                buckets[c][g][h] = (gs[m] - h * HALF, gd[m] - g * W, gw[m])

    # unified block count per (group, half): max over cores, >= 1 block per
    # group total so every group gets an eviction
    B = np.zeros((N_GROUPS, 2), np.int64)
    for g in range(N_GROUPS):
        for h in (0, 1):
            B[g, h] = max((buckets[c][g][h][0].shape[0] + P - 1) // P
                          for c in range(N_CORES))
        if B[g, 0] == 0 and B[g, 1] == 0:
            B[g, 0] = 1

    # pack consecutive groups into chunks of <= MAXBLK blocks (lo and hi
    # share one msg tile: lo runs occupy blocks [0, nblk_lo), hi the rest).
    # The first chunks are small so the first segment matmuls start early,
    # and the final chunk is small to shorten the conv tail.
    caps = [24, 48]
    chunks = []
    cur, cur_n = [], 0
    for g in range(N_GROUPS):
        nb = int(B[g, 0] + B[g, 1])
        cap = caps[len(chunks)] if len(chunks) < len(caps) else MAXBLK
        if cur and cur_n + nb > cap:
            chunks.append(cur)
            cur, cur_n = [], 0
        cur.append(g)
        cur_n += nb
    if cur:
        chunks.append(cur)
    # rebalance the tail: if the last chunk is large, split it
    if len(chunks) >= 2:
        last = chunks[-1]
        nlast = sum(int(B[g, 0] + B[g, 1]) for g in last)
        if nlast > 32 and len(last) >= 2:
            csum = 0
            for i in range(len(last) - 1, -1, -1):
                csum += int(B[last[i], 0] + B[last[i], 1])
                if csum >= 16:
                    break
            if i > 0:
                chunks[-1] = last[:i]
                chunks.append(last[i:])

    chunk_meta = []
    core_idx = [[] for _ in range(N_CORES)]
    core_sw = [[] for _ in range(N_CORES)]
    for groups in chunks:
        nblk_lo = int(sum(B[g, 0] for g in groups))
        nblk_hi = int(sum(B[g, 1] for g in groups))
        nblk = nblk_lo + nblk_hi
        runs = []
        b = 0
        for h in (0, 1):
            for g in groups:
                nb = int(B[g, h])
                if nb:
                    runs.append((g, h, b, b + nb))
                    b += nb
        chunk_meta.append(dict(nblk=nblk, nblk_lo=nblk_lo, runs=runs,
                               groups=list(groups)))

        for c in range(N_CORES):
            idx_flat = np.zeros(nblk * P, np.int16)
            seg_flat = np.full(nblk * P, -1.0, np.float32)
            w_flat = np.zeros(nblk * P, np.float32)
            for (g, h, b0, b1_) in runs:
                ids, segs, ws = buckets[c][g][h]
                n = ids.shape[0]
                o = b0 * P
                idx_flat[o:o + n] = ids.astype(np.int16)
                seg_flat[o:o + n] = segs.astype(np.float32)
                w_flat[o:o + n] = ws
            idx_t = np.tile(idx_flat.reshape(nblk * 8, 16).T, (8, 1))
            core_idx[c].append(idx_t)
            # S_w[e, s, b] = w * (seg == s); padded slots have seg=-1 -> zero
            seg_b = seg_flat.reshape(nblk, P)          # [b, e]
            w_b = w_flat.reshape(nblk, P)
            sw = (seg_b[:, :, None] == np.arange(W)[None, None, :])
            sw = sw * w_b[:, :, None]                  # [b, e, s]
            # device tile layout: [e(part), b, s]
            sw_t = np.ascontiguousarray(
                sw.transpose(1, 0, 2).astype(NP_BF16)) # [e, b, s]
            core_sw[c].append(sw_t.reshape(P, W * nblk))

    eidx = [np.ascontiguousarray(np.concatenate(core_idx[c], axis=1))
            for c in range(N_CORES)]
    esw = [np.ascontiguousarray(np.concatenate(core_sw[c], axis=1))
           for c in range(N_CORES)]
    return chunk_meta, eidx, esw


# ------------------------------------------------------------- device build --

def _build(metas, idx_cols, sw_cols):
    nc = bacc.Bacc(None, target_bir_lowering=False, num_devices=N_CORES,
                   num_swdge_queues=2)

    xT = nc.dram_tensor("xT", [IN_DIM, SHARD], BF16, kind="ExternalInput")
    w1T = nc.dram_tensor("w1T", [IN_DIM, HID], BF16, kind="ExternalInput")
    b1 = nc.dram_tensor("b1", [HID, 1], F32, kind="ExternalInput")
    w2T = nc.dram_tensor("w2T", [HID, HID], BF16, kind="ExternalInput")
    b2 = nc.dram_tensor("b2", [HID, 1], F32, kind="ExternalInput")
    conv_wT = nc.dram_tensor("conv_wT", [2, 2, HID, HID], BF16, kind="ExternalInput")
    conv_b = nc.dram_tensor("conv_b", [2, HID, 1], F32, kind="ExternalInput")
    headWT = nc.dram_tensor("headWT", [HID, 2 * LAT], BF16, kind="ExternalInput")
    head_b = nc.dram_tensor("head_b", [2 * LAT, 1], F32, kind="ExternalInput")
    eidx = nc.dram_tensor("eidx", [P, idx_cols], I16, kind="ExternalInput")
    esw = nc.dram_tensor("esw", [P, sw_cols], BF16, kind="ExternalInput")
    muv_out = nc.dram_tensor("muvT", [2 * LAT, SHARD], F32, kind="ExternalOutput")

    ag_in = [nc.dram_tensor(f"ag_in{i}", [SHARD, HID], BF16) for i in range(2)]
    tables = [nc.dram_tensor(f"h_full{i}", [N_NODES, HID], BF16,
                             addr_space="Shared") for i in range(2)]

    NT_FULL = SHARD // P            # 48 full 128-row publish tiles
    TAIL = SHARD - NT_FULL * P      # 106

    with TileContext(nc) as tc:
        with (
            tc.tile_pool(name="const", bufs=1) as cp,
            tc.tile_pool(name="big", bufs=1) as bigp,
            tc.tile_pool(name="work", bufs=3) as wp,
            tc.tile_pool(name="msgp", bufs=2) as msgp,
            tc.tile_pool(name="ps_tr", bufs=2, space="PSUM") as ps_tr,
        ):
            # ---- constants: keep SP/Act free (x loads + relus); use Pool
            w1t_sb = [cp.tile([P, HID], BF16, tag=f"w1_{k}", name=f"w1t_{k}") for k in range(4)]
            for k in range(4):
                nc.gpsimd.dma_start(out=w1t_sb[k][:], in_=w1T[k * P:(k + 1) * P, :])
            b1_sb = cp.tile([P, 1], F32, tag="b1")
            nc.gpsimd.dma_start(out=b1_sb[:], in_=b1[:, :])
            w2t_sb = cp.tile([P, HID], BF16, tag="w2")
            nc.gpsimd.dma_start(out=w2t_sb[:], in_=w2T[:, :])
            b2_sb = cp.tile([P, 1], F32, tag="b2")
            nc.gpsimd.dma_start(out=b2_sb[:], in_=b2[:, :])
            # conv-phase constants are emitted after the first AllGather so
            # their DMAs land in the collective window (see below)
            cw_sb = [[cp.tile([P, HID], BF16, tag=f"cw{l}{m}", name=f"cw_{l}_{m}") for m in range(2)]
                     for l in range(2)]
            cb_sb = [cp.tile([P, 1], F32, tag=f"cb{l}", name=f"cb_{l}") for l in range(2)]
            hw_sb = cp.tile([P, 2 * LAT], BF16, tag="hw")
            hb_sb = cp.tile([2 * LAT, 1], F32, tag="hb")
            ident = cp.tile([P, P], BF16, tag="ident")
            make_identity(nc, ident[:])

            hA = bigp.tile([P, SHARD], BF16, tag="hA")   # h2T, then h4T
            hB = bigp.tile([P, SHARD], BF16, tag="hB")   # h3T
            aggT = bigp.tile([P, SHARD], BF16, tag="aggT")
            rootT_sb = bigp.tile([P, SHARD], BF16, tag="rootT")
            rootT = [rootT_sb, rootT_sb]
            natf = [bigp.tile([P, NT_FULL, HID], BF16, tag=f"natf{i}",
                              name=f"natf_{i}") for i in range(2)]
            natt = [bigp.tile([P, HID], BF16, tag=f"natt{i}",
                              name=f"natt_{i}") for i in range(2)]
            muvT = bigp.tile([2 * LAT, SHARD], F32, tag="muvT")

            def emit_transpose_tiles(hT_tile, t_idx, n0, n1, eng_flip=None):
                # transpose hT[:, n0:n1] into natural-layout staging tiles;
                # PSUM->SBUF eviction on DVE (Act is busy with relus/evicts)
                t = n0 // P
                while n0 < n1:
                    w_ = min(P, n1 - n0)
                    tr_ps = ps_tr.tile([P, P], BF16, space="PSUM", tag="tr",
                                       name="trp")
                    nc.tensor.transpose(out=tr_ps[:w_, :],
                                        in_=hT_tile[:, n0:n0 + w_],
                                        identity=ident[:])
                    dst = natf[t_idx][:, t, :] if w_ == P else natt[t_idx][:TAIL, :]
                    nc.vector.tensor_copy(
                        out=dst[:w_, :] if w_ == P else dst,
                        in_=tr_ps[:w_, :])
                    n0 += w_
                    t += 1

            def emit_publish_piece(t_idx, c0, c1):
                # publish natf cols [c0, c1) as soon as their transposes land
                nc.sync.dma_start(
                    out=ag_in[t_idx][c0 * P:c1 * P, :].rearrange(
                        "(t r) h -> r t h", r=P),
                    in_=natf[t_idx][:, c0:c1, :])
                if c1 == NT_FULL:
                    nc.sync.dma_start(out=ag_in[t_idx][NT_FULL * P:, :],
                                      in_=natt[t_idx][:TAIL, :])

            PUB_AT = {4: (0, 16), 8: (16, 32), 13: (32, NT_FULL)}

            def maybe_publish(t_idx, done_tiles):
                if done_tiles in PUB_AT:
                    emit_publish_piece(t_idx, *PUB_AT[done_tiles])

            def emit_allgather(t_idx):
                nc.gpsimd.collective_compute(
                    "AllGather", mybir.AluOpType.bypass,
                    replica_groups=[list(range(N_CORES))],
                    ins=[ag_in[t_idx][:, :]],
                    outs=[tables[t_idx][:, :]],
                )

            # ---- MLP (bf16 matmuls, f32 psum) ----
            # software-pipelined so the PE stream has no dependency gaps:
            # stage t emits layer-1 matmuls for tile t, layer-2 for t-1, and
            # publish transposes for t-2
            cols = [sum(N_TILES[:i]) for i in range(len(N_TILES))]
            h1_sbs = {}
            # dedicated MLP PSUM pool (4 banks) scoped to the MLP so the
            # conv-phase pools can use the banks afterwards
            mlp_ps_cm = tc.tile_pool(name="mlp_ps", bufs=4, space="PSUM")
            mlp_ps = mlp_ps_cm.__enter__()

            def mlp_l1(t):
                nt, col = N_TILES[t], cols[t]
                # spread the 4 partition chunks over SP(3-in-1) and Pool(1)
                xt3 = wp.tile([P, 3, 512], BF16, tag="xt3")
                nc.sync.dma_start(
                    out=xt3[:, :, :nt],
                    in_=xT[0:3 * P, col:col + nt].rearrange(
                        "(k p) n -> p k n", p=P))
                xt1 = wp.tile([P, 512], BF16, tag="xt1")
                nc.gpsimd.dma_start(out=xt1[:, :nt],
                                    in_=xT[3 * P:4 * P, col:col + nt])
                h1_ps = mlp_ps.tile([P, 512], F32, space="PSUM", tag="lin")
                for k in range(4):
                    rhs = xt1[:, :nt] if k == 3 else xt3[:, k, :nt]
                    nc.tensor.matmul(out=h1_ps[:, :nt], lhsT=w1t_sb[k][:],
                                     rhs=rhs,
                                     start=(k == 0), stop=(k == 3))
                h1_sb = wp.tile([P, 512], BF16, tag="h1")
                nc.scalar.activation(out=h1_sb[:, :nt], in_=h1_ps[:, :nt],
                                     func=mybir.ActivationFunctionType.Relu,
                                     bias=b1_sb[:])
                h1_sbs[t] = h1_sb

            def mlp_l2(t):
                nt, col = N_TILES[t], cols[t]
                h2_ps = mlp_ps.tile([P, 512], F32, space="PSUM", tag="lin")
                nc.tensor.matmul(out=h2_ps[:, :nt], lhsT=w2t_sb[:],
                                 rhs=h1_sbs.pop(t)[:, :nt],
                                 start=True, stop=True)
                nc.scalar.activation(out=hA[:, col:col + nt], in_=h2_ps[:, :nt],
                                     func=mybir.ActivationFunctionType.Relu,
                                     bias=b2_sb[:])

            NTI = len(N_TILES)
            for t in range(NTI + 2):
                if t < NTI:
                    mlp_l1(t)
                if 1 <= t <= NTI:
                    mlp_l2(t - 1)
                if t >= 2:
                    emit_transpose_tiles(hA, 0, cols[t - 2],
                                         cols[t - 2] + N_TILES[t - 2])
                    maybe_publish(0, t - 1)
            mlp_ps_cm.__exit__(None, None, None)
            ps_lin_cm = tc.tile_pool(name="ps_lin", bufs=3, space="PSUM")
            ps_lin = ps_lin_cm.__enter__()
            ps_agg_cm = tc.tile_pool(name="ps_agg", bufs=3, space="PSUM")
            ps_agg = ps_agg_cm.__enter__()
            emit_allgather(0)

            # conv-phase constant loads land here, inside the AG0 window
            for l in range(2):
                for m in range(2):
                    nc.gpsimd.dma_start(out=cw_sb[l][m][:], in_=conv_wT[l, m, :, :])
                nc.gpsimd.dma_start(out=cb_sb[l][:], in_=conv_b[l, :, :])
            nc.gpsimd.dma_start(out=hw_sb[:], in_=headWT[:, :])
            nc.gpsimd.dma_start(out=hb_sb[:], in_=head_b[:, :])

            def emit_root(layer, hT_in):
                col = 0
                for nt in N_TILES:
                    ps = ps_lin.tile([P, 512], F32, space="PSUM", tag="lin")
                    nc.tensor.matmul(out=ps[:, :nt], lhsT=cw_sb[layer][1][:],
                                     rhs=hT_in[:, col:col + nt],
                                     start=True, stop=True)
                    nc.scalar.activation(out=rootT[layer][:, col:col + nt],
                                         in_=ps[:, :nt],
                                         func=mybir.ActivationFunctionType.Copy)
                    col += nt

            def conv_layer(layer, hT_in, hT_out, table, pub_idx=None,
                           tile_tail=None):
                # aggregation: chunked gathers + per-group PE segment sums
                icol = 0
                scol = 0
                done_g = 0          # groups fully evicted so far
                done_tiles = 0      # linear tiles emitted so far

                def emit_ready_linear(done_g, done_tiles, force=False):
                    # emit linear tiles whose agg columns are complete
                    avail = min(done_g * W, SHARD)
                    col = done_tiles * 512
                    while done_tiles < len(N_TILES):
                        nt = N_TILES[done_tiles]
                        if col + nt > avail and not force:
                            break
                        ps = ps_lin.tile([P, 512], F32, space="PSUM", tag="lin")
                        nc.tensor.matmul(out=ps[:, :nt], lhsT=cw_sb[layer][0][:],
                                         rhs=aggT[:, col:col + nt],
                                         start=True, stop=False)
                        nc.tensor.matmul(out=ps[:, :nt], lhsT=ident[:],
                                         rhs=rootT[layer][:, col:col + nt],
                                         start=False, stop=True)
                        nc.scalar.activation(
                            out=hT_out[:, col:col + nt], in_=ps[:, :nt],
                            func=mybir.ActivationFunctionType.Relu,
                            bias=cb_sb[layer][:])
                        if pub_idx is not None:
                            emit_transpose_tiles(hT_out, pub_idx, col, col + nt)
                        if tile_tail is not None:
                            tile_tail(col, nt)
                        col += nt
                        done_tiles += 1
                        if pub_idx is not None:
                            maybe_publish(pub_idx, done_tiles)
                    return done_tiles

                for ci, meta in enumerate(metas):
                    nblk, nblk_lo = meta["nblk"], meta["nblk_lo"]
                    qn = 0
                    idx_t = wp.tile([P, MAXBLK * 8], I16, tag="eidx")
                    nc.scalar.dma_start(out=idx_t[:, :nblk * 8],
                                        in_=eidx[:, icol:icol + nblk * 8])
                    # S_w is a large transfer; DMAs hold the issuing engine,
                    # so it lives on SP (idle during the conv phase)
                    s_w = msgp.tile([P, MAXBLK, W], BF16, tag="sw")
                    nc.sync.dma_start(
                        out=s_w[:, :nblk, :],
                        in_=esw[:, scol:scol + W * nblk].rearrange(
                            "p (b s) -> p b s", s=W))

                    msg = msgp.tile([P, MAXBLK, HID], BF16, tag="msg")
                    if nblk_lo:
                        nc.gpsimd.dma_gather(
                            out_ap=msg[:, :nblk_lo, :], in_ap=table[:HALF, :],
                            idxs_ap=idx_t[:, :nblk_lo * 8],
                            num_idxs=nblk_lo * P, num_idxs_reg=nblk_lo * P,
                            elem_size=HID, single_packet=False,
                            queue_num=qn)
                    if nblk - nblk_lo:
                        nh = nblk - nblk_lo
                        nc.gpsimd.dma_gather(
                            out_ap=msg[:, nblk_lo:nblk, :], in_ap=table[HALF:, :],
                            idxs_ap=idx_t[:, nblk_lo * 8:nblk * 8],
                            num_idxs=nh * P, num_idxs_reg=nh * P,
                            elem_size=HID, single_packet=False,
                            queue_num=qn)

                    # one psum + one eviction per group: a group's lo and hi
                    # runs accumulate into the same tile
                    by_group = {}
                    for (g, h, b0, b1_) in meta["runs"]:
                        by_group.setdefault(g, []).append((h, b0, b1_))
                    for g in meta["groups"]:
                        ps = ps_agg.tile([P, W], F32, space="PSUM", tag="agg")
                        blocks = [(h, b) for (h, b0, b1_) in by_group[g]
                                  for b in range(b0, b1_)]
                        for i, (h, b) in enumerate(blocks):
                            nc.tensor.matmul(out=ps[:], lhsT=msg[:, b, :],
                                             rhs=s_w[:, b, :],
                                             start=(i == 0),
                                             stop=(i == len(blocks) - 1))
                        gw = min(W, SHARD - g * W)
                        nc.scalar.activation(
                            out=aggT[:, g * W:g * W + gw], in_=ps[:, :gw],
                            func=mybir.ActivationFunctionType.Copy)
                    icol += nblk * 8
                    scol += W * nblk
                    done_g += len(meta["groups"])
                    done_tiles = emit_ready_linear(done_g, done_tiles)
                done_tiles = emit_ready_linear(done_g, done_tiles, force=True)

            emit_root(0, hA)
            conv_layer(0, hA, hB, tables[0], pub_idx=1)
            emit_allgather(1)
            emit_root(1, hB)

            # ---- heads fused into conv2's linear phase ----
            def head_tail(col, nt):
                ps = ps_lin.tile([2 * LAT, 512], F32, space="PSUM", tag="lin",
                                 name="headps")
                nc.tensor.matmul(out=ps[:, :nt], lhsT=hw_sb[:],
                                 rhs=hA[:, col:col + nt], start=True, stop=True)
                nc.vector.tensor_tensor(
                    out=muvT[:, col:col + nt], in0=ps[:, :nt],
                    in1=hb_sb[:].to_broadcast([2 * LAT, nt]),
                    op=mybir.AluOpType.add)
                # stream the output tile-by-tile so there is no DMA tail
                eng = nc.sync if (col // 512) % 2 == 0 else nc.scalar
                eng.dma_start(out=muv_out[:, col:col + nt],
                              in_=muvT[:, col:col + nt])

            conv_layer(1, hB, hA, tables[1], tile_tail=head_tail)
            ps_agg_cm.__exit__(None, None, None)
            ps_lin_cm.__exit__(None, None, None)

    nc.finalize()
    return nc


# -------------------------------------------------------------------- driver --

def _get_compiled(x, edge_index, edge_attr, weights):
    src = np.asarray(edge_index[0]).astype(np.int64)
    dst = np.asarray(edge_index[1]).astype(np.int64)
    wgt = np.asarray(edge_attr, dtype=np.float32)
    x = np.asarray(x, dtype=np.float32)

    per_core_edges = []
    for c in range(N_CORES):
        sel = (dst >= c * SHARD) & (dst < (c + 1) * SHARD)
        s, d, wv = src[sel], dst[sel] - c * SHARD, wgt[sel]
        order = np.argsort(d, kind="stable")
        per_core_edges.append((s[order], d[order], wv[order]))

    metas, eidx, esw = _unified_structure(per_core_edges)
    idx_cols = sum(m["nblk"] * 8 for m in metas)
    sw_cols = sum(m["nblk"] * W for m in metas)

    nc = _build(metas, idx_cols, sw_cols)

    (W1, b1, W2, b2, g1_rel_W, g1_rel_b, g1_root_W,
     g2_rel_W, g2_rel_b, g2_root_W, mu_W, mu_b, lv_W, lv_b) = [
        np.asarray(w, dtype=np.float32) for w in weights]

    conv_wT = np.stack([
        np.stack([g1_rel_W.T, g1_root_W.T]),
        np.stack([g2_rel_W.T, g2_root_W.T]),
    ]).astype(NP_BF16).copy()
    conv_b = np.stack([g1_rel_b[:, None], g2_rel_b[:, None]]).copy()
    headWT = np.ascontiguousarray(
        np.concatenate([mu_W, lv_W], axis=0).T.astype(NP_BF16))
    head_b = np.concatenate([mu_b, lv_b])[:, None].copy()

    common = dict(
        w1T=np.ascontiguousarray(W1.T.astype(NP_BF16)), b1=b1[:, None].copy(),
        w2T=np.ascontiguousarray(W2.T.astype(NP_BF16)), b2=b2[:, None].copy(),
        conv_wT=conv_wT, conv_b=conv_b, headWT=headWT, head_b=head_b,
    )
    in_maps = []
    for c in range(N_CORES):
        m = dict(common)
        m["xT"] = np.ascontiguousarray(x[c * SHARD:(c + 1) * SHARD, :].T.astype(NP_BF16))
        m["eidx"] = eidx[c]
        m["esw"] = esw[c]
        in_maps.append(m)
    return nc, in_maps


def kernel(x, edge_index, edge_attr,
           W1, b1, W2, b2,
           g1_rel_W, g1_rel_b, g1_root_W,
           g2_rel_W, g2_rel_b, g2_root_W,
           mu_W, mu_b, lv_W, lv_b):
    weights = (W1, b1, W2, b2, g1_rel_W, g1_rel_b, g1_root_W,
               g2_rel_W, g2_rel_b, g2_root_W, mu_W, mu_b, lv_W, lv_b)
    nc, in_maps = _get_compiled(x, edge_index, edge_attr, weights)
    res = bass_utils.run_bass_kernel_spmd(nc, in_maps,
                                          core_ids=list(range(N_CORES)))
    muvT = np.concatenate([res.results[c]["muvT"] for c in range(N_CORES)],
                          axis=1)
    return (np.ascontiguousarray(muvT[:LAT, :].T),
            np.ascontiguousarray(muvT[LAT:, :].T))
